# revision 57
# baseline (speedup 1.0000x reference)
"""Trainium2 Bass kernel for a dense transformer block (pre-LN, MHA + GELU MLP).

Problem shapes (hardcoded): x [2, 2048, 768] f32, mask [2, 2048] int32,
12 heads x 64 dims, hidden 3072.

Sharding: 8 cores = (batch b in {0,1}) x (query shard s in {0..3}).
Each core gets its 512-query slice of x (xqT) plus the HOST-COMPACTED set
of unmasked key tokens of its batch (xkT, padded to NK=1152). Masked keys
(~half of all tokens) never reach the device: K/V projection, scores, exp
and attnV all shrink by ~44% vs processing all 2048 keys. Padding keys are
zero (=> v rows 0) and their softmax-denominator entry is zeroed via m01.

On-chip layout is feature-major: activations are [features, tokens], every
matmul contracts over the partition dim. LN gain/bias are folded into the
next matmul's weights host-side; per-token mean/rstd come from ones-vector
matmuls (partition reduction on PE) and are broadcast back across
partitions with gpsimd partition_broadcast.

fp8 (e4m3, x32 weight scale) with DoubleRow is used for qkv, fc1 and fc2
matmuls. Softmax denominators come free from a WS-scaled ones column
appended to V (M=65 attnV matmul); the WS of the V weights cancels against
it so the V epilogue is a pure f32->fp8 cast. Max-subtraction is skipped:
|scores| <= ~4 by construction, so exp cannot overflow.
"""

import numpy as np
import ml_dtypes

import concourse.bass as bass
import concourse.tile as tile
import concourse.mybir as mybir
from concourse import bacc
from concourse.bass import ts
from concourse.bass_utils import run_bass_kernel_spmd
from concourse.alu_op_type import AluOpType

BF16 = mybir.dt.bfloat16
F32 = mybir.dt.float32
FP8 = mybir.dt.float8e4
DR = mybir.MatmulPerfMode.DoubleRow
WS = 32.0   # fp8 weight scale (dodges e4m3 subnormals)

B = 2
N = 2048
D = 768
H = 12
HD = 64
HID = 3072
EPS = 1e-5
SCALE = HD ** -0.5
NQ = 512          # queries per core
NSH = N // NQ     # query shards per batch
NC = B * NSH      # 8 cores
C6 = D // 128     # feature chunks
NK = 1152         # padded compacted key count per batch
K9 = NK // 128    # key chunks
HO24 = HID // 128
# key tiles for LN stats / apply / K-proj free dim (offset, width)
KT = [(0, 512), (512, 512), (1024, 128)]

AF = mybir.ActivationFunctionType
OP = AluOpType

_cached = {}
_rid = [0]


def _build_nc(sbp):
    nc = bacc.Bacc("TRN2", target_bir_lowering=False, debug=False,
                   enable_asserts=False, num_devices=NC)

    xqT = nc.dram_tensor("xqT", [D, NQ], BF16, kind="ExternalInput").ap()
    xkT = nc.dram_tensor("xkT", [D, NK], BF16, kind="ExternalInput").ap()
    wqkv = nc.dram_tensor("wqkv", [D, 3 * D], FP8, kind="ExternalInput").ap()
    wproj = nc.dram_tensor("wproj", [D, D], BF16, kind="ExternalInput").ap()
    wfc1 = nc.dram_tensor("wfc1", [D, HID], FP8, kind="ExternalInput").ap()
    wfc2 = nc.dram_tensor("wfc2", [HID, D], BF16, kind="ExternalInput").ap()
    bqkv = nc.dram_tensor("bqkv", [128, 18], F32, kind="ExternalInput").ap()
    bproj = nc.dram_tensor("bproj", [128, 6], F32, kind="ExternalInput").ap()
    bfc1 = nc.dram_tensor("bfc1", [128, 24], F32, kind="ExternalInput").ap()
    bfc2 = nc.dram_tensor("bfc2", [128, 6], F32, kind="ExternalInput").ap()
    uproj = nc.dram_tensor("uproj", [128, 6], BF16, kind="ExternalInput").ap()
    mask01 = nc.dram_tensor("mask01", [128, K9], F32, kind="ExternalInput").ap()
    out_d = nc.dram_tensor("out", [D, NQ], F32, kind="ExternalOutput").ap()
    import os
    dbg = {}
    if os.environ.get("KDBG"):
        dbg["y"] = nc.dram_tensor("dbg_y", [D, NQ], F32, kind="ExternalOutput").ap()
        dbg["x2"] = nc.dram_tensor("dbg_x2", [D, NQ], F32, kind="ExternalOutput").ap()
        dbg["xnq"] = nc.dram_tensor("dbg_xnq", [D, NQ], F32, kind="ExternalOutput").ap()
        dbg["xnk"] = nc.dram_tensor("dbg_xnk", [D, NK], F32, kind="ExternalOutput").ap()
        dbg["q"] = nc.dram_tensor("dbg_q", [D, NQ], F32, kind="ExternalOutput").ap()
        dbg["k"] = nc.dram_tensor("dbg_k", [D, NK], F32, kind="ExternalOutput").ap()
        dbg["yu"] = nc.dram_tensor("dbg_yu", [H, HD + 1, NQ], F32, kind="ExternalOutput").ap()

    with tile.TileContext(nc) as tc:
        _body(nc, tc, sbp, xqT, xkT, wqkv, wproj, wfc1, wfc2, bqkv, bproj,
              bfc1, bfc2, uproj, mask01, out_d, dbg)
    nc.compile()
    return nc


def _body(nc, tc, sbp, xqT, xkT, wqkv_d, wproj_d, wfc1_d, wfc2_d, bqkv_d,
          bproj_d, bfc1_d, bfc2_d, uproj_d, mask01_d, out_d, dbg=None):
    dbg = dbg or {}
    P1_cm = tc.tile_pool(name="p1", bufs=1); P1 = P1_cm.__enter__()
    P2_cm = tc.tile_pool(name="p2", bufs=2); P2 = P2_cm.__enter__()
    P4_cm = tc.tile_pool(name="p4", bufs=4); P4 = P4_cm.__enter__()
    P6_cm = tc.tile_pool(name="p6", bufs=6); P6 = P6_cm.__enter__()
    ps_mm_cm = tc.tile_pool(name="ps_mm", bufs=4, space="PSUM")
    ps_mm = ps_mm_cm.__enter__()
    ps_sc_cm = tc.tile_pool(name="ps_sc", bufs=2, space="PSUM")
    ps_sc = ps_sc_cm.__enter__()
    dr_cm = tc.tile_pool(name="drp", bufs=8, space="DRAM")
    drp = dr_cm.__enter__()

    def bcast_dma(dst, src_row):
        """replicate a single-partition row across dst partitions:
        SBUF row -> DRAM scratch -> partition-broadcast DMA back"""
        dt = drp.tile([1, 512], src_row.dtype, tag="dr",
                      name=f"dr{_rid[0]}")[:, 0:src_row.free_size()]
        _rid[0] += 1
        nc.sync.dma_start(out=dt, in_=src_row)
        rap = bass.AP(tensor=dt.tensor, offset=dt.offset,
                      ap=[[0, dst.shape[0]]] + [list(x) for x in dt.ap[1:]])
        nc.sync.dma_start(out=dst, in_=rap)

    # ---- constants ----
    ones = P1.tile([128, 128], BF16, tag="ones")
    nc.vector.memset(ones, 1.0)
    ones_col = ones[:, 0:1]
    m01 = P1.tile([128, K9], F32, tag="m01")
    nc.sync.dma_start(out=m01, in_=mask01_d)
    bqkv_s = P1.tile([128, 18], F32, tag="bqkv")
    nc.sync.dma_start(out=bqkv_s, in_=bqkv_d)
    bproj_s = P1.tile([128, 6], F32, tag="bproj")
    nc.sync.dma_start(out=bproj_s, in_=bproj_d)
    bfc1_s = P1.tile([128, 24], F32, tag="bfc1")
    nc.sync.dma_start(out=bfc1_s, in_=bfc1_d)
    bfc2_s = P1.tile([128, 6], F32, tag="bfc2")
    nc.sync.dma_start(out=bfc2_s, in_=bfc2_d)
    uproj_s = P1.tile([128, 6], BF16, tag="uproj")
    nc.sync.dma_start(out=uproj_s, in_=uproj_d)
    eps1 = P1.tile([1, 1], F32, tag="eps1")
    nc.vector.memset(eps1, EPS)
    sumx0 = P1.tile([1, NQ], BF16, tag="sumx0")

    # ---- activations / weights to SBUF ----
    # xk / wqkv / wfc1 / wfc2a / wfc2b have disjoint lifetimes: rotate them
    # through 3 pool slots (xk dies after LN1 apply -> wfc2a takes its slot;
    # wqkv dies after the last K projection -> wfc2b takes its slot)
    xq_sb = P1.tile([128, C6, NQ], BF16, tag="xq")
    xqr = xqT.rearrange("(a p) q -> p a q", p=128)
    for c in range(C6):
        nc.sync.dma_start(out=xq_sb[:, c, :], in_=xqr[:, c, :])
    xk_sb = P2.tile([128, C6, NK], BF16, tag="wbig", bufs=3)
    xkr = xkT.rearrange("(a p) q -> p a q", p=128)
    for c in range(C6):
        for hf in range(2):
            nc.sync.dma_start(out=xk_sb[:, c, ts(hf, NK // 2)],
                              in_=xkr[:, c, ts(hf, NK // 2)])
    w_sb = P2.tile([128, C6, 3 * D], FP8, tag="wbig", bufs=3)
    wqr = wqkv_d.rearrange("(a p) o -> p a o", p=128)
    for c in range(C6):
        nc.sync.dma_start(out=w_sb[:, c, :], in_=wqr[:, c, :])
    wfc1_sb = P2.tile([128, C6, HID], FP8, tag="wbig", bufs=3)
    w1r = wfc1_d.rearrange("(a p) o -> p a o", p=128)
    for c in range(C6):
        nc.sync.dma_start(out=wfc1_sb[:, c, :], in_=w1r[:, c, :])
    wproj_sb = P1.tile([128, C6, D], BF16, tag="wproj")
    wpr = wproj_d.rearrange("(a p) o -> p a o", p=128)
    for c in range(C6):
        nc.sync.dma_start(out=wproj_sb[:, c, :], in_=wpr[:, c, :])
    w2r = wfc2_d.rearrange("(a p) o -> p a o", p=128)

    xnq = P1.tile([128, C6, NQ], FP8, tag="xnq")
    xnk = P1.tile([128, C6, NK], FP8, tag="xnk")

    # ========== LN1 ==========
    def emit_stats(src3, off, w, is_q, t):
        ps_sum = ps_mm.tile([1, 512], F32, tag="mm", name=f"pssum{t}")
        ps_sq = ps_mm.tile([1, 512], F32, tag="mm", name=f"pssq{t}")
        for c in range(C6):
            xa = src3[:, c, off:off + w]
            sq = P4.tile([128, 512], BF16, tag="tmp", name=f"sq{t}_{c}")
            nc.vector.tensor_tensor(sq[:, 0:w], xa, xa, op=OP.mult)
            nc.tensor.matmul(ps_sum[:, 0:w], ones_col, xa,
                             start=(c == 0), stop=(c == C6 - 1))
            nc.tensor.matmul(ps_sq[:, 0:w], ones_col, sq[:, 0:w],
                             start=(c == 0), stop=(c == C6 - 1))
        if is_q:
            nc.vector.tensor_copy(out=sumx0, in_=ps_sum[:, 0:w])
        mrow = P6.tile([1, 512], BF16, tag="mrow", name=f"mrow{t}", bufs=2)
        nc.scalar.activation(mrow[:, 0:w], ps_sum[:, 0:w], AF.Copy, scale=1.0 / D)
        m2 = P6.tile([1, 512], F32, tag="r", name=f"m2_{t}", bufs=4)
        nc.scalar.activation(m2[:, 0:w], ps_sum[:, 0:w], AF.Square, scale=1.0 / D)
        vrow = P6.tile([1, 512], F32, tag="vrow", name=f"vrow{t}", bufs=2)
        nc.vector.scalar_tensor_tensor(out=vrow[:, 0:w], in0=ps_sq[:, 0:w],
                                       scalar=1.0 / D, in1=m2[:, 0:w],
                                       op0=OP.mult, op1=OP.subtract)
        srt = P6.tile([1, 512], F32, tag="r", name=f"srt{t}", bufs=4)
        nc.scalar.activation(srt[:, 0:w], vrow[:, 0:w], AF.Sqrt, bias=eps1)
        rf = P6.tile([1, 512], F32, tag="r", name=f"rf{t}", bufs=4)
        nc.vector.reciprocal_approx_fast(out=rf[:, 0:w], in_=srt[:, 0:w])
        rrow = P6.tile([1, 512], BF16, tag="rrow", name=f"rrow{t}", bufs=2)
        nc.vector.tensor_copy(out=rrow[:, 0:w], in_=rf[:, 0:w])
        mbs = P4.tile([128, 512], BF16, tag="bcs", name=f"mbs{t}", bufs=8)
        nc.gpsimd.partition_broadcast(mbs[:, 0:w], mrow[:, 0:w])
        rbs_ = P4.tile([128, 512], BF16, tag="bcs", name=f"rbs_{t}", bufs=8)
        nc.gpsimd.partition_broadcast(rbs_[:, 0:w], rrow[:, 0:w])
        return mbs, rbs_

    def emit_stats_rows(t, w, srcs):
        ps_sum, ps_sq = srcs
        mrow = P6.tile([1, 512], BF16, tag="mrow", name=f"mrowL{t}", bufs=2)
        nc.scalar.activation(mrow[:, 0:w], ps_sum[:, 0:w], AF.Copy,
                             scale=1.0 / D)
        m2 = P6.tile([1, 512], F32, tag="r", name=f"m2L{t}", bufs=4)
        nc.scalar.activation(m2[:, 0:w], ps_sum[:, 0:w], AF.Square,
                             scale=1.0 / D)
        vrow = P6.tile([1, 512], F32, tag="vrow", name=f"vrowL{t}", bufs=2)
        nc.vector.scalar_tensor_tensor(out=vrow[:, 0:w], in0=ps_sq[:, 0:w],
                                       scalar=1.0 / D, in1=m2[:, 0:w],
                                       op0=OP.mult, op1=OP.subtract)
        srt = P6.tile([1, 512], F32, tag="r", name=f"srtL{t}", bufs=4)
        nc.scalar.activation(srt[:, 0:w], vrow[:, 0:w], AF.Sqrt, bias=eps1)
        rf = P6.tile([1, 512], F32, tag="r", name=f"rfL{t}", bufs=4)
        nc.vector.reciprocal_approx_fast(out=rf[:, 0:w], in_=srt[:, 0:w])
        rrow = P6.tile([1, 512], BF16, tag="rrow", name=f"rrowL{t}", bufs=2)
        nc.vector.tensor_copy(out=rrow[:, 0:w], in_=rf[:, 0:w])
        mbs = P4.tile([128, 512], BF16, tag="bcs", name=f"mbsL{t}", bufs=8)
        nc.gpsimd.partition_broadcast(mbs[:, 0:w], mrow[:, 0:w])
        rbs_ = P4.tile([128, 512], BF16, tag="bcs", name=f"rbsL{t}", bufs=8)
        nc.gpsimd.partition_broadcast(rbs_[:, 0:w], rrow[:, 0:w])
        return mbs, rbs_

    def emit_stats_pe(src3, off, w, t):
        ps_sum = ps_mm.tile([1, 512], F32, tag="mm", name=f"pssumL{t}")
        ps_sq = ps_mm.tile([1, 512], F32, tag="mm", name=f"pssqL{t}")
        for c in range(C6):
            xa = src3[:, c, off:off + w]
            sq = P4.tile([128, 512], BF16, tag="tmp", name=f"sqL{t}_{c}")
            nc.vector.tensor_tensor(sq[:, 0:w], xa, xa, op=OP.mult)
            nc.tensor.matmul(ps_sum[:, 0:w], ones_col, xa,
                             start=(c == 0), stop=(c == C6 - 1))
            nc.tensor.matmul(ps_sq[:, 0:w], ones_col, sq[:, 0:w],
                             start=(c == 0), stop=(c == C6 - 1))
        return ps_sum, ps_sq

    def emit_apply(dst, src3, off, w, mbs, rbs_, t):
        for c in range(C6):
            d = P4.tile([128, 512], BF16, tag="tmp", name=f"d{t}_{c}")
            nc.vector.tensor_tensor(d[:, 0:w], src3[:, c, off:off + w],
                                    mbs[:, 0:w], op=OP.subtract)
            nc.vector.tensor_tensor(dst[:, c, off:off + w], d[:, 0:w],
                                    rbs_[:, 0:w], op=OP.mult)

    # ---- attention data structures / helpers ----
    # vsb per key chunk: 12 heads x (64 v-dims + 1 denom col), padded to 784
    vsb = P1.tile([128, K9, 16 * ((H * (HD + 1) + 15) // 16)], FP8, tag="vsb")
    m01r = bass.AP(tensor=m01.tensor, offset=m01.offset,
                   ap=[list(m01.ap[0]), list(m01.ap[1]), [0, H], [0, 1]])
    vsb_h = vsb[:, :, 0:H * (HD + 1)].rearrange("p k (h e) -> p k h e", e=HD + 1)
    nc.vector.tensor_copy(out=vsb_h[:, :, :, HD:HD + 1], in_=m01r)

    kch_state = [None]

    def emit_k_mm(kch_p, p, ti, ci):
        off, w = KT[ti]
        if ci == 0:
            kch_state[0] = ps_mm.tile([128, 512], F32, tag="mm",
                                      name=f"kst{_rid[0]}")
            _rid[0] += 1
        nc.tensor.matmul(kch_state[0][:, 0:w], w_sb[:, ci:ci + 2, ts(6 + p, 128)],
                         xnk[:, ci:ci + 2, off:off + w],
                         start=(ci == 0), stop=(ci == C6 - 2), perf_mode=DR)
        if ci == C6 - 2:
            nc.vector.tensor_scalar(out=kch_p[:, off:off + w],
                                    in0=kch_state[0][:, 0:w],
                                    scalar1=1.0 / WS,
                                    scalar2=bqkv_s[:, 6 + p:6 + p + 1],
                                    op0=OP.mult, op1=OP.add)
            kch_state[0] = None

    def emit_v_chunk(tk):
        # psum keeps the WS weight scale; it cancels against the WS-scaled
        # denominator column, so this epilogue is a pure f32->fp8 cast
        for half in range(2):
            psv = ps_mm.tile([128, 512], F32, tag="mm", name=f"psv{tk}_{half}")
            for ci in range(0, C6, 2):
                nc.tensor.matmul(psv[:, 0:384],
                                 xnk[:, ci:ci + 2, ts(tk, 128)],
                                 w_sb[:, ci:ci + 2, 12 * 128 + half * 384:
                                      12 * 128 + (half + 1) * 384],
                                 start=(ci == 0), stop=(ci == C6 - 2),
                                 perf_mode=DR)
            vout = vsb[:, tk, half * 390:half * 390 + 390].rearrange(
                "p (h e) -> p h e", e=HD + 1)[:, :, 0:HD]
            nc.vector.tensor_copy(
                out=vout, in_=psv[:, 0:384].rearrange("p (h d) -> p h d", h=6))

    def emit_attnv_pair(p, q, ex2t, ps_y2):
        for j in range(2):
            h = 2 * p + j
            nc.tensor.matmul(ps_y2[j][0:HD + 1, :],
                             vsb[:, 2 * q:2 * q + 2, h * 65:h * 65 + 65],
                             ex2t[:, :, j, :],
                             start=(q == 0), stop=False, perf_mode=DR)

    def emit_attnv_single(p, ex1, ps_y2):
        for j in range(2):
            h = 2 * p + j
            nc.tensor.matmul(ps_y2[j][0:HD + 1, :],
                             vsb[:, K9 - 1, h * 65:h * 65 + 65],
                             ex1[:, j, :],
                             start=False, stop=True)

    def emit_recips(p, ps_y2):
        r65s = []
        for j in range(2):
            if "yu" in dbg:
                du_ = P4.tile([128, 512], F32, tag="dbgt", name=f"dyu{p}_{j}", bufs=1)
                nc.vector.tensor_copy(out=du_[0:HD + 1, :],
                                      in_=ps_y2[j][0:HD + 1, :])
                nc.sync.dma_start(out=dbg["yu"][2 * p + j, :, :],
                                  in_=du_[0:HD + 1, :])
            sr = P6.tile([1, 512], F32, tag="rr", name=f"sr{p}_{j}", bufs=2)
            nc.vector.tensor_copy(out=sr, in_=ps_y2[j][HD:HD + 1, :])
            rr = P6.tile([1, 512], F32, tag="rr", name=f"rr{p}_{j}", bufs=2)
            nc.vector.reciprocal_approx_fast(out=rr, in_=sr)
            rrb = P6.tile([1, 512], BF16, tag="rrb", name=f"rrb{p}_{j}", bufs=2)
            nc.vector.tensor_copy(out=rrb, in_=rr)
            rbf = P4.tile([128, 512], BF16, tag="tf", name=f"rbf{p}_{j}")
            nc.gpsimd.partition_broadcast(rbf[0:HD, :], rrb)
            r65s.append(rbf)
        return r65s

    def emit_deferred_epilogue(p, ps_y2, r65s):
        for j in range(2):
            ps_y = ps_y2[j]
            if j == 0:
                nc.vector.tensor_tensor(y_sb[0:HD, p, :], ps_y[0:HD, :],
                                        r65s[j][0:HD, :], op=OP.mult)
            else:
                yt = P4.tile([128, 512], BF16, tag="tmp", name=f"yt{p}")
                nc.vector.tensor_tensor(yt[0:HD, :], ps_y[0:HD, :],
                                        r65s[j][0:HD, :], op=OP.mult)
                # partition-shift copy via the Act DMA queue: never stuck
                # behind a bulk weight transfer on the SP rings
                nc.scalar.dma_start(out=y_sb[HD:128, p, :], in_=yt[0:HD, :])

    y_sb = P1.tile([128, C6, NQ], BF16, tag="y")
    qT = P1.tile([128, C6, NQ], BF16, tag="qT")
    kch = {}
    pend = []
    exs = []

    # ---- LN1 stats / apply / Q / K0-tile0, interleaved so the in-order PE
    # queue never waits on a row chain that a later-queued matmul doesn't
    # need: stats of the next tile fill PE while the previous tile's rows,
    # broadcast and apply run on ACT/DVE/pool ----
    def emit_q(co):
        ps = ps_mm.tile([128, 512], F32, tag="mm", name=f"qps{co}")
        for ci in range(0, C6, 2):
            nc.tensor.matmul(ps, w_sb[:, ci:ci + 2, ts(co, 128)],
                             xnq[:, ci:ci + 2, :],
                             start=(ci == 0), stop=(ci == C6 - 2), perf_mode=DR)
        nc.vector.tensor_scalar(out=qT[:, co, :], in0=ps,
                                scalar1=1.0 / WS,
                                scalar2=bqkv_s[:, co:co + 1],
                                op0=OP.mult, op1=OP.add)

    # all four stats tiles first (their ACT rows stay ahead of the exp
    # stream so the Sqrt->Exp table switch happens exactly once), then the
    # minimal prefix scores tk0..3 need: apply-q, Q chunk 0, apply-k0,
    # K0 tile 0. Everything else (V, K rest, Q chunks 1..5) streams inside
    # the p-loop where PE otherwise idles.
    # k1/k2 row chains wait on PE stats that finish last; keeping them out
    # of the DVE queue until after apply-q/Q/apply-k0/K0 lets those (whose
    # inputs are ready much earlier) run without head-of-line blocking
    stq = emit_stats(xq_sb, 0, NQ, True, 0)
    stk = [None, None, None]
    stk[0] = emit_stats(xk_sb, KT[0][0], KT[0][1], False, 1)
    psk1 = emit_stats_pe(xk_sb, KT[1][0], KT[1][1], 2)
    psk2 = emit_stats_pe(xk_sb, KT[2][0], KT[2][1], 3)
    emit_apply(xnq, xq_sb, 0, NQ, stq[0], stq[1], 0)
    emit_q(0)
    emit_apply(xnk, xk_sb, KT[0][0], KT[0][1], stk[0][0], stk[0][1], 1)
    kch[0] = P2.tile([128, NK], BF16, tag="kch", name="kch0")
    for ci in range(0, C6, 2):
        emit_k_mm(kch[0], 0, 0, ci)
    stk[1] = emit_stats_rows(2, KT[1][1], psk1)
    emit_apply(xnk, xk_sb, KT[1][0], KT[1][1], stk[1][0], stk[1][1], 2)
    stk[2] = emit_stats_rows(3, KT[2][1], psk2)
    emit_apply(xnk, xk_sb, KT[2][0], KT[2][1], stk[2][0], stk[2][1], 3)
    warm = P6.tile([1, 512], F32, tag="r", bufs=4, name="warm")
    nc.scalar.activation(warm[0:1, 0:8], m01[0:1, 0:8], AF.Exp)

    if "xnq" in dbg:
        for c in range(C6):
            dt_ = P4.tile([128, 512], F32, tag="dbgt", name=f"dbxnq{c}", bufs=1)
            nc.vector.tensor_copy(out=dt_, in_=xnq[:, c, :])
            nc.sync.dma_start(out=dbg["xnq"][ts(c, 128), :], in_=dt_)
    if "xnk" in dbg:
        for c in range(C6):
            for i, (off, w) in enumerate(KT):
                dt_ = P4.tile([128, 512], F32, tag="dbgt",
                              name=f"dbxnk{c}_{i}", bufs=1)
                nc.vector.tensor_copy(out=dt_[:, 0:w],
                                      in_=xnk[:, c, off:off + w])
                nc.sync.dma_start(out=dbg["xnk"][ts(c, 128), off:off + w],
                                  in_=dt_[:, 0:w])
    if "q" in dbg:
        for c in range(C6):
            dq_ = P4.tile([128, 512], F32, tag="dbgt", name=f"dbq{c}", bufs=1)
            nc.vector.tensor_copy(out=dq_, in_=qT[:, c, :])
            nc.sync.dma_start(out=dbg["q"][ts(c, 128), :], in_=dq_)

    # ========== attention pipeline ==========
    # proj co=0 and u.y chains for p==5's spare PE slots (y chunks 0..4)
    pp0 = [None]
    ps_us = [None]

    def emit_proj_unit(which, c):
        if which == 0:
            if pp0[0] is None:
                pp0[0] = ps_mm.tile([128, 512], F32, tag="mm", name="prj0")
            nc.tensor.matmul(pp0[0], wproj_sb[:, c, ts(0, 128)], y_sb[:, c, :],
                             start=(c == 0), stop=False)
        else:
            if ps_us[0] is None:
                ps_us[0] = ps_mm.tile([1, 512], F32, tag="mm", name="ps_us")
            nc.tensor.matmul(ps_us[0], uproj_s[:, c:c + 1], y_sb[:, c, :],
                             start=(c == 0), stop=False)

    wfc2a = wfc2b = None
    for p in range(C6):
        if p == 1:
            # fc2 first half into xk's freed slot; small chunks so the
            # attention-phase bounce DMAs never queue behind a big transfer
            wfc2a = P2.tile([128, 12, D], BF16, tag="wbig", bufs=3)
            for g in range(12):
                nc.sync.dma_start(out=wfc2a[:, g, :], in_=w2r[:, g, :])
        if p == C6 - 1:
            # fc2 second half into wqkv's freed slot
            wfc2b = P2.tile([128, 12, D], BF16, tag="wbig", bufs=3)
            for g in range(12):
                nc.sync.dma_start(out=wfc2b[:, g, :], in_=w2r[:, 12 + g, :])
        if p < C6 - 1:
            kch[p + 1] = P2.tile([128, NK], BF16, tag="kch", name=f"kch{p + 1}")
            kwork = [(p + 1, ti, ci) for ti in range(3)
                     for ci in range(0, C6, 2)]
            if p == 0:
                # K0 tiles 1,2 still pending; K1 follows
                kwork = [(0, ti, ci) for ti in (1, 2)
                         for ci in range(0, C6, 2)] + kwork
        else:
            kwork = []
        vwork = list(range(K9)) if p == 0 else []
        qwork = [p + 1] if p < C6 - 1 else []
        pwork = [(w_, c) for c in range(5) for w_ in range(2)] if p == C6 - 1 \
            else []
        kper = 2 if p == 0 else 1
        ex = {}
        ps_y2 = [None, None]
        for tk in range(K9):
            pss = ps_sc.tile([128, 2, 512], F32, tag="sc")
            for j in range(2):
                po = j * 64
                nc.tensor.matmul(pss[:, j, :],
                                 kch[p][po:po + 64, ts(tk, 128)],
                                 qT[po:po + 64, p, :],
                                 start=True, stop=True)
            if tk % 2 == 0:
                ex[tk // 2] = P6.tile([128, 2, 2, 512], FP8, tag="exp",
                                      name=f"ex_{p}_{tk // 2}", bufs=5)
            nc.scalar.activation(ex[tk // 2][:, tk % 2, :, :], pss, AF.Exp)
            if vwork:
                emit_v_chunk(vwork.pop(0))
            if tk == 0 and pend:
                # previous p's last attnV chunk + softmax denominators: the
                # exp this waits on is long done, so no ACT bubble here
                emit_attnv_single(pend[-1]["p"], exs.pop(0), pend[-1]["ps_y2"])
                pend[-1]["r65s"] = emit_recips(pend[-1]["p"],
                                               pend[-1]["ps_y2"])
            if tk == 2 and pend:
                emit_deferred_epilogue(**pend.pop())
            if tk == 3:
                ps_y2[0] = ps_mm.tile([128, 512], F32, tag="mm", name=f"psyA{p}")
                ps_y2[1] = ps_mm.tile([128, 512], F32, tag="mm", name=f"psyB{p}")
                emit_attnv_pair(p, 0, ex.pop(0), ps_y2)
            if tk in (4, 6, 8):
                q = (tk - 2) // 2
                emit_attnv_pair(p, q, ex.pop(q), ps_y2)
            for _ in range(kper):
                if kwork:
                    kp, ti, ci = kwork.pop(0)
                    emit_k_mm(kch[kp], kp, ti, ci)
            if qwork and tk == 1:
                emit_q(qwork.pop())
            if pwork and tk >= 4:
                for _ in range(2):
                    w_, c = pwork.pop(0)
                    emit_proj_unit(w_, c)
        while kwork:
            kp, ti, ci = kwork.pop(0)
            emit_k_mm(kch[kp], kp, ti, ci)
        exs.append(ex.pop(4)[:, 0, :, :])
        pend.append(dict(p=p, ps_y2=ps_y2, r65s=None))
        if "k" in dbg:
            for i, (off, w) in enumerate(KT):
                dk_ = P4.tile([128, 512], F32, tag="dbgt", name=f"dbk{p}_{i}", bufs=1)
                nc.vector.tensor_copy(out=dk_[:, 0:w], in_=kch[p][:, off:off + w])
                nc.sync.dma_start(out=dbg["k"][ts(p, 128), off:off + w],
                                  in_=dk_[:, 0:w])

    # ========== proj tail + LN2 ==========
    # warm the Sqrt act table now (last exp already issued; load hides
    # under the proj matmuls)
    x2_sb = P1.tile([128, C6, NQ], F32, tag="x2")
    xn2_sb = P1.tile([128, C6, NQ], FP8, tag="xn2")
    warms = P6.tile([1, 512], F32, tag="r", bufs=4, name="warms")
    nc.scalar.activation(warms[0:1, 0:8], m01[0:1, 0:8], AF.Sqrt)
    # proj co=1,2 partial chains (need only y0..y4) fill PE while p5's
    # denominators resolve; they live in the score-psum banks, now free
    pc12 = {}
    for co in (1, 2):
        psx = ps_sc.tile([128, 512], F32, tag="sc", name=f"pcs{co}")
        for c in range(5):
            nc.tensor.matmul(psx, wproj_sb[:, c, ts(co, 128)], y_sb[:, c, :],
                             start=(c == 0), stop=False)
        pc12[co] = psx
        if co == 1:
            emit_attnv_single(pend[-1]["p"], exs.pop(0), pend[-1]["ps_y2"])
            pend[-1]["r65s"] = emit_recips(pend[-1]["p"], pend[-1]["ps_y2"])
    emit_deferred_epilogue(**pend.pop())
    if "y" in dbg:
        for c in range(C6):
            dy_ = P4.tile([128, 512], F32, tag="dbgt", name=f"dby{c}", bufs=1)
            nc.vector.tensor_copy(out=dy_, in_=y_sb[:, c, :])
            nc.sync.dma_start(out=dbg["y"][ts(c, 128), :], in_=dy_)
    nc.tensor.matmul(pp0[0], wproj_sb[:, 5, ts(0, 128)], y_sb[:, 5, :],
                     start=False, stop=True)
    nc.tensor.matmul(ps_us[0], uproj_s[:, 5:6], y_sb[:, 5, :],
                     start=False, stop=True)
    for co in (1, 2):
        nc.tensor.matmul(pc12[co], wproj_sb[:, 5, ts(co, 128)], y_sb[:, 5, :],
                         start=False, stop=True)
    nc.vector.scalar_tensor_tensor(out=x2_sb[:, 0, :], in0=pp0[0],
                                   scalar=bproj_s[:, 0:1],
                                   in1=xq_sb[:, 0, :], op0=OP.add, op1=OP.add)
    # squares on ACT (idle after the last exp; Square lives in every act
    # table set). LN2 row chain part 1 interleaves on ACT after sq0.
    sq0 = P4.tile([128, 512], BF16, tag="tmp", name="sq2_0")
    nc.scalar.activation(sq0, x2_sb[:, 0, :], AF.Square)
    srow = P6.tile([1, 512], F32, tag="r", bufs=4)
    nc.vector.scalar_tensor_tensor(out=srow, in0=ps_us[0], scalar=float(sbp),
                                   in1=sumx0, op0=OP.add, op1=OP.add)
    mrow2 = P6.tile([1, 512], BF16, tag="r", bufs=4)
    nc.scalar.activation(mrow2, srow, AF.Copy, scale=1.0 / D)
    m22 = P6.tile([1, 512], F32, tag="r", bufs=4)
    nc.scalar.activation(m22, srow, AF.Square, scale=1.0 / D)
    mbs2 = P4.tile([128, 512], BF16, tag="bcs", bufs=8)
    nc.gpsimd.partition_broadcast(mbs2, mrow2)
    d2l = [None] * C6
    sql = [sq0]
    ps_sq2 = ps_mm.tile([1, 512], F32, tag="mm")
    for co in (1, 2):
        nc.vector.scalar_tensor_tensor(out=x2_sb[:, co, :], in0=pc12[co],
                                       scalar=bproj_s[:, co:co + 1],
                                       in1=xq_sb[:, co, :],
                                       op0=OP.add, op1=OP.add)
        sq = P4.tile([128, 512], BF16, tag="tmp", name=f"sq2_{co}")
        nc.scalar.activation(sq, x2_sb[:, co, :], AF.Square)
        sql.append(sq)
    # remaining proj chunks; per-chunk: residual epilogue, square, and the
    # sumsq accumulation (behind by one, so PE never waits ACT)
    for co in range(3, C6):
        ps = ps_mm.tile([128, 512], F32, tag="mm")
        for c in range(C6):
            nc.tensor.matmul(ps, wproj_sb[:, c, ts(co, 128)], y_sb[:, c, :],
                             start=(c == 0), stop=(c == C6 - 1))
        nc.tensor.matmul(ps_sq2, ones_col, sql[co - 3],
                         start=(co == 3), stop=False)
        nc.vector.scalar_tensor_tensor(out=x2_sb[:, co, :], in0=ps,
                                       scalar=bproj_s[:, co:co + 1],
                                       in1=xq_sb[:, co, :],
                                       op0=OP.add, op1=OP.add)
        sq = P4.tile([128, 512], BF16, tag="tmp", name=f"sq2_{co}")
        nc.scalar.activation(sq, x2_sb[:, co, :], AF.Square)
        sql.append(sq)
        c = co - 3
        d2l[c] = P4.tile([128, 512], BF16, tag="bcs", name=f"d2_{c}", bufs=8)
        nc.vector.tensor_tensor(d2l[c], x2_sb[:, c, :], mbs2, op=OP.subtract)
    for c in range(3, C6):
        nc.tensor.matmul(ps_sq2, ones_col, sql[c],
                         start=False, stop=(c == C6 - 1))

    if "x2" in dbg:
        for c in range(C6):
            dx2_ = P4.tile([128, 512], F32, tag="dbgt", name=f"dbx2{c}", bufs=1)
            nc.vector.tensor_copy(out=dx2_, in_=x2_sb[:, c, :])
            nc.sync.dma_start(out=dbg["x2"][ts(c, 128), :], in_=dx2_)
    # ========== LN2 rest of row chain ==========
    vrow2 = P6.tile([1, 512], F32, tag="r", bufs=4)
    nc.vector.scalar_tensor_tensor(out=vrow2, in0=ps_sq2, scalar=1.0 / D,
                                   in1=m22, op0=OP.mult, op1=OP.subtract)
    srt2 = P6.tile([1, 512], F32, tag="r", bufs=4)
    nc.scalar.activation(srt2, vrow2, AF.Sqrt, bias=eps1)
    warm2 = P6.tile([1, 512], F32, tag="r", bufs=4, name="warm2")
    nc.scalar.activation(warm2[0:1, 0:8], srt2[0:1, 0:8], AF.Gelu)
    rf2 = P6.tile([1, 512], F32, tag="r", bufs=4)
    nc.vector.reciprocal_approx_fast(out=rf2, in_=srt2)
    rrow2 = P6.tile([1, 512], BF16, tag="r", bufs=4)
    nc.vector.tensor_copy(out=rrow2, in_=rf2)
    for c in (3, 4, 5):
        d2l[c] = P4.tile([128, 512], BF16, tag="bcs", name=f"d2_{c}", bufs=8)
        nc.vector.tensor_tensor(d2l[c], x2_sb[:, c, :], mbs2, op=OP.subtract)
    rbs2 = P4.tile([128, 512], BF16, tag="bcs", bufs=8)
    nc.gpsimd.partition_broadcast(rbs2, rrow2)
    for c in range(C6):
        nc.vector.tensor_tensor(xn2_sb[:, c, :], d2l[c], rbs2, op=OP.mult)

    # ========== MLP (fc1 fp8 DoubleRow, fc2 bf16) ==========
    h_sb = P1.tile([128, HO24, NQ], BF16, tag="h")
    for ho in range(HO24):
        ps = ps_mm.tile([128, 512], F32, tag="mm")
        for ci in range(0, C6, 2):
            nc.tensor.matmul(ps, wfc1_sb[:, ci:ci + 2, ts(ho, 128)],
                             xn2_sb[:, ci:ci + 2, :],
                             start=(ci == 0), stop=(ci == C6 - 2), perf_mode=DR)
        nc.scalar.activation(h_sb[:, ho, :], ps, AF.Gelu,
                             scale=1.0 / WS, bias=bfc1_s[:, ho:ho + 1])
    for co in range(C6):
        ps = ps_mm.tile([128, 512], F32, tag="mm")
        for ho in range(HO24):
            wsrc = wfc2a if ho < 12 else wfc2b
            nc.tensor.matmul(ps, wsrc[:, ho % 12, ts(co, 128)], h_sb[:, ho, :],
                             start=(ho == 0), stop=(ho == HO24 - 1))
        o = P2.tile([128, 512], F32, tag="ot", name=f"o_{co}")
        nc.vector.scalar_tensor_tensor(out=o, in0=ps,
                                       scalar=bfc2_s[:, co:co + 1],
                                       in1=x2_sb[:, co, :],
                                       op0=OP.add, op1=OP.add)
        nc.sync.dma_start(out=out_d[ts(co, 128), :], in_=o)

    for cm in (dr_cm, ps_sc_cm, ps_mm_cm, P6_cm, P4_cm, P2_cm, P1_cm):
        cm.__exit__(None, None, None)


def _host_prep(x, mask, ln1_g, ln1_b, qkv_w, proj_w, proj_b, ln2_g, ln2_b,
               fc1_w, fc1_b, fc2_w, fc2_b):
    bf = ml_dtypes.bfloat16
    f8 = ml_dtypes.float8_e4m3
    f32 = np.float32
    x = np.asarray(x, f32)
    mask = np.asarray(mask)
    qkv_w = np.asarray(qkv_w, f32)
    proj_w = np.asarray(proj_w, f32)
    fc1_w = np.asarray(fc1_w, f32)
    fc2_w = np.asarray(fc2_w, f32)
    ln1_g = np.asarray(ln1_g, f32); ln1_b = np.asarray(ln1_b, f32)
    ln2_g = np.asarray(ln2_g, f32); ln2_b = np.asarray(ln2_b, f32)
    proj_b = np.asarray(proj_b, f32)
    fc1_b = np.asarray(fc1_b, f32); fc2_b = np.asarray(fc2_b, f32)

    wqkv_f = qkv_w * ln1_g[None, :]
    bqkv_f = qkv_w @ ln1_b
    wqkv_f[0:D] *= SCALE
    bqkv_f[0:D] *= SCALE
    bv = bqkv_f[2 * D:3 * D].copy()
    bqkv_f[2 * D:3 * D] = 0.0     # v bias folded into proj bias (sum(attn)=1)
    bproj_f = proj_b + proj_w @ bv
    wfc1_f = fc1_w * ln2_g[None, :]
    bfc1_f = fc1_w @ ln2_b + fc1_b

    shared = {
        "wqkv": np.ascontiguousarray(wqkv_f.T * WS).astype(f8),
        "wproj": np.ascontiguousarray(proj_w.T).astype(bf),
        "wfc1": np.ascontiguousarray(wfc1_f.T * WS).astype(f8),
        "wfc2": np.ascontiguousarray(fc2_w.T).astype(bf),
        "bqkv": np.ascontiguousarray(bqkv_f.reshape(18, 128).T).astype(f32),
        "bproj": np.ascontiguousarray(bproj_f.reshape(6, 128).T).astype(f32),
        "bfc1": np.ascontiguousarray(bfc1_f.reshape(24, 128).T).astype(f32),
        "bfc2": np.ascontiguousarray(fc2_b.reshape(6, 128).T).astype(f32),
        "uproj": np.ascontiguousarray(
            proj_w.sum(axis=0).reshape(6, 128).T).astype(bf),
    }
    sbp = float(bproj_f.sum())

    # per-batch compacted key set (host-side gather of unmasked tokens)
    xk_b, m01_b = [], []
    for b in range(B):
        idx = np.nonzero(mask[b] != 1)[0]
        nk = len(idx)
        assert nk <= NK, f"batch {b}: {nk} unmasked keys > NK={NK}"
        xk = np.zeros((NK, D), f32)
        xk[:nk] = x[b][idx]
        m01 = np.zeros((NK,), f32)
        m01[:nk] = WS   # cancels the WS carried by the V weights
        xk_b.append(np.ascontiguousarray(xk.T).astype(bf))
        m01_b.append(np.ascontiguousarray(m01.reshape(K9, 128).T).astype(f32))

    in_maps = []
    for core in range(NC):
        b, s = divmod(core, NSH)
        im = dict(shared)
        im["xqT"] = np.ascontiguousarray(
            x[b, s * NQ:(s + 1) * NQ].T).astype(bf)
        im["xkT"] = xk_b[b]
        im["mask01"] = m01_b[b]
        in_maps.append(im)
    return in_maps, sbp


def kernel(**inputs):
    in_maps, sbp = _host_prep(**inputs)
    if _cached.get("sbp") != sbp:
        _cached["nc"] = _build_nc(sbp)
        _cached["sbp"] = sbp
    res = run_bass_kernel_spmd(_cached["nc"], in_maps, core_ids=list(range(NC)))
    out = np.empty((B, N, D), np.float32)
    for core in range(NC):
        b, s = divmod(core, NSH)
        out[b, s * NQ:(s + 1) * NQ, :] = res.results[core]["out"].T
    return out


# revision 58
# speedup vs baseline: 1.0175x; 1.0175x over previous
"""Trainium2 Bass kernel for a dense transformer block (pre-LN, MHA + GELU MLP).

Problem shapes (hardcoded): x [2, 2048, 768] f32, mask [2, 2048] int32,
12 heads x 64 dims, hidden 3072.

Sharding: 8 cores = (batch b in {0,1}) x (query shard s in {0..3}).
Each core gets its 512-query slice of x (xqT) plus the HOST-COMPACTED set
of unmasked key tokens of its batch (xkT, padded to NK=1152). Masked keys
(~half of all tokens) never reach the device: K/V projection, scores, exp
and attnV all shrink by ~44% vs processing all 2048 keys. Padding keys are
zero (=> v rows 0) and their softmax-denominator entry is zeroed via m01.

On-chip layout is feature-major: activations are [features, tokens], every
matmul contracts over the partition dim. LN gain/bias are folded into the
next matmul's weights host-side; per-token mean/rstd come from ones-vector
matmuls (partition reduction on PE) and are broadcast back across
partitions with gpsimd partition_broadcast.

fp8 (e4m3, x32 weight scale) with DoubleRow is used for qkv, fc1 and fc2
matmuls. Softmax denominators come free from a WS-scaled ones column
appended to V (M=65 attnV matmul); the WS of the V weights cancels against
it so the V epilogue is a pure f32->fp8 cast. Max-subtraction is skipped:
|scores| <= ~4 by construction, so exp cannot overflow.
"""

import numpy as np
import ml_dtypes

import concourse.bass as bass
import concourse.tile as tile
import concourse.mybir as mybir
from concourse import bacc
from concourse.bass import ts
from concourse.bass_utils import run_bass_kernel_spmd
from concourse.alu_op_type import AluOpType

BF16 = mybir.dt.bfloat16
F32 = mybir.dt.float32
FP8 = mybir.dt.float8e4
DR = mybir.MatmulPerfMode.DoubleRow
WS = 32.0   # fp8 weight scale (dodges e4m3 subnormals)

B = 2
N = 2048
D = 768
H = 12
HD = 64
HID = 3072
EPS = 1e-5
SCALE = HD ** -0.5
NQ = 512          # queries per core
NSH = N // NQ     # query shards per batch
NC = B * NSH      # 8 cores
C6 = D // 128     # feature chunks
NK = 1152         # padded compacted key count per batch
K9 = NK // 128    # key chunks
HO24 = HID // 128
# key tiles for LN stats / apply / K-proj free dim (offset, width)
KT = [(0, 512), (512, 512), (1024, 128)]

AF = mybir.ActivationFunctionType
OP = AluOpType

_cached = {}
_rid = [0]


def _build_nc(sbp):
    nc = bacc.Bacc("TRN2", target_bir_lowering=False, debug=False,
                   enable_asserts=False, num_devices=NC)

    xqT = nc.dram_tensor("xqT", [D, NQ], BF16, kind="ExternalInput").ap()
    xkT = nc.dram_tensor("xkT", [D, NK], BF16, kind="ExternalInput").ap()
    wqkv = nc.dram_tensor("wqkv", [D, 3 * D], FP8, kind="ExternalInput").ap()
    wproj = nc.dram_tensor("wproj", [D, D], BF16, kind="ExternalInput").ap()
    wfc1 = nc.dram_tensor("wfc1", [D, HID], FP8, kind="ExternalInput").ap()
    wfc2 = nc.dram_tensor("wfc2", [HID, D], BF16, kind="ExternalInput").ap()
    bqkv = nc.dram_tensor("bqkv", [128, 18], F32, kind="ExternalInput").ap()
    bproj = nc.dram_tensor("bproj", [128, 6], F32, kind="ExternalInput").ap()
    bfc1 = nc.dram_tensor("bfc1", [128, 24], F32, kind="ExternalInput").ap()
    bfc2 = nc.dram_tensor("bfc2", [128, 6], F32, kind="ExternalInput").ap()
    uproj = nc.dram_tensor("uproj", [128, 6], BF16, kind="ExternalInput").ap()
    mask01 = nc.dram_tensor("mask01", [128, K9], F32, kind="ExternalInput").ap()
    out_d = nc.dram_tensor("out", [D, NQ], F32, kind="ExternalOutput").ap()
    import os
    dbg = {}
    if os.environ.get("KDBG"):
        dbg["y"] = nc.dram_tensor("dbg_y", [D, NQ], F32, kind="ExternalOutput").ap()
        dbg["x2"] = nc.dram_tensor("dbg_x2", [D, NQ], F32, kind="ExternalOutput").ap()
        dbg["xnq"] = nc.dram_tensor("dbg_xnq", [D, NQ], F32, kind="ExternalOutput").ap()
        dbg["xnk"] = nc.dram_tensor("dbg_xnk", [D, NK], F32, kind="ExternalOutput").ap()
        dbg["q"] = nc.dram_tensor("dbg_q", [D, NQ], F32, kind="ExternalOutput").ap()
        dbg["k"] = nc.dram_tensor("dbg_k", [D, NK], F32, kind="ExternalOutput").ap()
        dbg["yu"] = nc.dram_tensor("dbg_yu", [H, HD + 1, NQ], F32, kind="ExternalOutput").ap()

    with tile.TileContext(nc) as tc:
        _body(nc, tc, sbp, xqT, xkT, wqkv, wproj, wfc1, wfc2, bqkv, bproj,
              bfc1, bfc2, uproj, mask01, out_d, dbg)
    nc.compile()
    return nc


def _body(nc, tc, sbp, xqT, xkT, wqkv_d, wproj_d, wfc1_d, wfc2_d, bqkv_d,
          bproj_d, bfc1_d, bfc2_d, uproj_d, mask01_d, out_d, dbg=None):
    dbg = dbg or {}
    P1_cm = tc.tile_pool(name="p1", bufs=1); P1 = P1_cm.__enter__()
    P2_cm = tc.tile_pool(name="p2", bufs=2); P2 = P2_cm.__enter__()
    P4_cm = tc.tile_pool(name="p4", bufs=4); P4 = P4_cm.__enter__()
    P6_cm = tc.tile_pool(name="p6", bufs=6); P6 = P6_cm.__enter__()
    ps_mm_cm = tc.tile_pool(name="ps_mm", bufs=4, space="PSUM")
    ps_mm = ps_mm_cm.__enter__()
    ps_sc_cm = tc.tile_pool(name="ps_sc", bufs=2, space="PSUM")
    ps_sc = ps_sc_cm.__enter__()
    dr_cm = tc.tile_pool(name="drp", bufs=8, space="DRAM")
    drp = dr_cm.__enter__()

    def bcast_dma(dst, src_row):
        """replicate a single-partition row across dst partitions:
        SBUF row -> DRAM scratch -> partition-broadcast DMA back"""
        dt = drp.tile([1, 512], src_row.dtype, tag="dr",
                      name=f"dr{_rid[0]}")[:, 0:src_row.free_size()]
        _rid[0] += 1
        nc.sync.dma_start(out=dt, in_=src_row)
        rap = bass.AP(tensor=dt.tensor, offset=dt.offset,
                      ap=[[0, dst.shape[0]]] + [list(x) for x in dt.ap[1:]])
        nc.sync.dma_start(out=dst, in_=rap)

    # ---- constants ----
    ones = P1.tile([128, 128], BF16, tag="ones")
    nc.vector.memset(ones, 1.0)
    ones_col = ones[:, 0:1]
    m01 = P1.tile([128, K9], F32, tag="m01")
    nc.sync.dma_start(out=m01, in_=mask01_d)
    bqkv_s = P1.tile([128, 18], F32, tag="bqkv")
    nc.sync.dma_start(out=bqkv_s, in_=bqkv_d)
    bproj_s = P1.tile([128, 6], F32, tag="bproj")
    nc.sync.dma_start(out=bproj_s, in_=bproj_d)
    bfc1_s = P1.tile([128, 24], F32, tag="bfc1")
    nc.sync.dma_start(out=bfc1_s, in_=bfc1_d)
    bfc2_s = P1.tile([128, 6], F32, tag="bfc2")
    nc.sync.dma_start(out=bfc2_s, in_=bfc2_d)
    uproj_s = P1.tile([128, 6], BF16, tag="uproj")
    nc.sync.dma_start(out=uproj_s, in_=uproj_d)
    eps1 = P1.tile([1, 1], F32, tag="eps1")
    nc.vector.memset(eps1, EPS)
    sumx0 = P1.tile([1, NQ], BF16, tag="sumx0")

    # ---- activations / weights to SBUF ----
    # xk / wqkv / wfc1 / wfc2a / wfc2b have disjoint lifetimes: rotate them
    # through 3 pool slots (xk dies after LN1 apply -> wfc2a takes its slot;
    # wqkv dies after the last K projection -> wfc2b takes its slot)
    xq_sb = P1.tile([128, C6, NQ], BF16, tag="xq")
    xqr = xqT.rearrange("(a p) q -> p a q", p=128)
    for c in range(C6):
        nc.sync.dma_start(out=xq_sb[:, c, :], in_=xqr[:, c, :])
    xk_sb = P2.tile([128, C6, NK], BF16, tag="wbig", bufs=3)
    xkr = xkT.rearrange("(a p) q -> p a q", p=128)
    for c in range(C6):
        for hf in range(2):
            nc.sync.dma_start(out=xk_sb[:, c, ts(hf, NK // 2)],
                              in_=xkr[:, c, ts(hf, NK // 2)])
    w_sb = P2.tile([128, C6, 3 * D], FP8, tag="wbig", bufs=3)
    wqr = wqkv_d.rearrange("(a p) o -> p a o", p=128)
    for c in range(C6):
        nc.sync.dma_start(out=w_sb[:, c, :], in_=wqr[:, c, :])
    wfc1_sb = P2.tile([128, C6, HID], FP8, tag="wbig", bufs=3)
    w1r = wfc1_d.rearrange("(a p) o -> p a o", p=128)
    for c in range(C6):
        nc.sync.dma_start(out=wfc1_sb[:, c, :], in_=w1r[:, c, :])
    wproj_sb = P1.tile([128, C6, D], BF16, tag="wproj")
    wpr = wproj_d.rearrange("(a p) o -> p a o", p=128)
    for c in range(C6):
        nc.sync.dma_start(out=wproj_sb[:, c, :], in_=wpr[:, c, :])
    w2r = wfc2_d.rearrange("(a p) o -> p a o", p=128)

    xnq = P1.tile([128, C6, NQ], FP8, tag="xnq")
    xnk = P1.tile([128, C6, NK], FP8, tag="xnk")

    # ========== LN1 ==========
    def emit_stats(src3, off, w, is_q, t):
        ps_sum = ps_mm.tile([1, 512], F32, tag="mm", name=f"pssum{t}")
        ps_sq = ps_mm.tile([1, 512], F32, tag="mm", name=f"pssq{t}")
        for c in range(C6):
            xa = src3[:, c, off:off + w]
            sq = P4.tile([128, 512], BF16, tag="tmp", name=f"sq{t}_{c}")
            nc.vector.tensor_tensor(sq[:, 0:w], xa, xa, op=OP.mult)
            nc.tensor.matmul(ps_sum[:, 0:w], ones_col, xa,
                             start=(c == 0), stop=(c == C6 - 1))
            nc.tensor.matmul(ps_sq[:, 0:w], ones_col, sq[:, 0:w],
                             start=(c == 0), stop=(c == C6 - 1))
        if is_q:
            nc.vector.tensor_copy(out=sumx0, in_=ps_sum[:, 0:w])
        mrow = P6.tile([1, 512], BF16, tag="mrow", name=f"mrow{t}", bufs=2)
        nc.scalar.activation(mrow[:, 0:w], ps_sum[:, 0:w], AF.Copy, scale=1.0 / D)
        m2 = P6.tile([1, 512], F32, tag="r", name=f"m2_{t}", bufs=4)
        nc.scalar.activation(m2[:, 0:w], ps_sum[:, 0:w], AF.Square, scale=1.0 / D)
        vrow = P6.tile([1, 512], F32, tag="vrow", name=f"vrow{t}", bufs=2)
        nc.vector.scalar_tensor_tensor(out=vrow[:, 0:w], in0=ps_sq[:, 0:w],
                                       scalar=1.0 / D, in1=m2[:, 0:w],
                                       op0=OP.mult, op1=OP.subtract)
        srt = P6.tile([1, 512], F32, tag="r", name=f"srt{t}", bufs=4)
        nc.scalar.activation(srt[:, 0:w], vrow[:, 0:w], AF.Sqrt, bias=eps1)
        rf = P6.tile([1, 512], F32, tag="r", name=f"rf{t}", bufs=4)
        nc.vector.reciprocal_approx_fast(out=rf[:, 0:w], in_=srt[:, 0:w])
        rrow = P6.tile([1, 512], BF16, tag="rrow", name=f"rrow{t}", bufs=2)
        nc.vector.tensor_copy(out=rrow[:, 0:w], in_=rf[:, 0:w])
        mbs = P4.tile([128, 512], BF16, tag="bcs", name=f"mbs{t}", bufs=8)
        nc.gpsimd.partition_broadcast(mbs[:, 0:w], mrow[:, 0:w])
        rbs_ = P4.tile([128, 512], BF16, tag="bcs", name=f"rbs_{t}", bufs=8)
        nc.gpsimd.partition_broadcast(rbs_[:, 0:w], rrow[:, 0:w])
        return mbs, rbs_

    def emit_apply(dst, src3, off, w, mbs, rbs_, t):
        for c in range(C6):
            d = P4.tile([128, 512], BF16, tag="tmp", name=f"d{t}_{c}")
            nc.vector.tensor_tensor(d[:, 0:w], src3[:, c, off:off + w],
                                    mbs[:, 0:w], op=OP.subtract)
            nc.vector.tensor_tensor(dst[:, c, off:off + w], d[:, 0:w],
                                    rbs_[:, 0:w], op=OP.mult)

    # ---- attention data structures / helpers ----
    # vsb per key chunk: 12 heads x (64 v-dims + 1 denom col), padded to 784
    vsb = P1.tile([128, K9, 16 * ((H * (HD + 1) + 15) // 16)], FP8, tag="vsb")
    m01r = bass.AP(tensor=m01.tensor, offset=m01.offset,
                   ap=[list(m01.ap[0]), list(m01.ap[1]), [0, H], [0, 1]])
    vsb_h = vsb[:, :, 0:H * (HD + 1)].rearrange("p k (h e) -> p k h e", e=HD + 1)
    nc.vector.tensor_copy(out=vsb_h[:, :, :, HD:HD + 1], in_=m01r)

    kch_state = [None]

    def emit_k_mm(kch_p, p, ti, ci):
        off, w = KT[ti]
        if ci == 0:
            kch_state[0] = ps_mm.tile([128, 512], F32, tag="mm",
                                      name=f"kst{_rid[0]}")
            _rid[0] += 1
        nc.tensor.matmul(kch_state[0][:, 0:w], w_sb[:, ci:ci + 2, ts(6 + p, 128)],
                         xnk[:, ci:ci + 2, off:off + w],
                         start=(ci == 0), stop=(ci == C6 - 2), perf_mode=DR)
        if ci == C6 - 2:
            nc.vector.tensor_scalar(out=kch_p[:, off:off + w],
                                    in0=kch_state[0][:, 0:w],
                                    scalar1=1.0 / WS,
                                    scalar2=bqkv_s[:, 6 + p:6 + p + 1],
                                    op0=OP.mult, op1=OP.add)
            kch_state[0] = None

    def emit_v_chunk(tk):
        # psum keeps the WS weight scale; it cancels against the WS-scaled
        # denominator column, so this epilogue is a pure f32->fp8 cast
        for half in range(2):
            psv = ps_mm.tile([128, 512], F32, tag="mm", name=f"psv{tk}_{half}")
            for ci in range(0, C6, 2):
                nc.tensor.matmul(psv[:, 0:384],
                                 xnk[:, ci:ci + 2, ts(tk, 128)],
                                 w_sb[:, ci:ci + 2, 12 * 128 + half * 384:
                                      12 * 128 + (half + 1) * 384],
                                 start=(ci == 0), stop=(ci == C6 - 2),
                                 perf_mode=DR)
            vout = vsb[:, tk, half * 390:half * 390 + 390].rearrange(
                "p (h e) -> p h e", e=HD + 1)[:, :, 0:HD]
            nc.vector.tensor_copy(
                out=vout, in_=psv[:, 0:384].rearrange("p (h d) -> p h d", h=6))

    def emit_attnv_pair(p, q, ex2t, ps_y2):
        for j in range(2):
            h = 2 * p + j
            nc.tensor.matmul(ps_y2[j][0:HD + 1, :],
                             vsb[:, 2 * q:2 * q + 2, h * 65:h * 65 + 65],
                             ex2t[:, :, j, :],
                             start=(q == 0), stop=False, perf_mode=DR)

    def emit_attnv_single(p, ex1, ps_y2):
        for j in range(2):
            h = 2 * p + j
            nc.tensor.matmul(ps_y2[j][0:HD + 1, :],
                             vsb[:, K9 - 1, h * 65:h * 65 + 65],
                             ex1[:, j, :],
                             start=False, stop=True)

    def emit_recips(p, ps_y2):
        r65s = []
        for j in range(2):
            if "yu" in dbg:
                du_ = P4.tile([128, 512], F32, tag="dbgt", name=f"dyu{p}_{j}", bufs=1)
                nc.vector.tensor_copy(out=du_[0:HD + 1, :],
                                      in_=ps_y2[j][0:HD + 1, :])
                nc.sync.dma_start(out=dbg["yu"][2 * p + j, :, :],
                                  in_=du_[0:HD + 1, :])
            sr = P6.tile([1, 512], F32, tag="rr", name=f"sr{p}_{j}", bufs=2)
            nc.vector.tensor_copy(out=sr, in_=ps_y2[j][HD:HD + 1, :])
            rr = P6.tile([1, 512], F32, tag="rr", name=f"rr{p}_{j}", bufs=2)
            nc.vector.reciprocal_approx_fast(out=rr, in_=sr)
            rrb = P6.tile([1, 512], BF16, tag="rrb", name=f"rrb{p}_{j}", bufs=2)
            nc.vector.tensor_copy(out=rrb, in_=rr)
            rbf = P4.tile([128, 512], BF16, tag="tf", name=f"rbf{p}_{j}")
            nc.gpsimd.partition_broadcast(rbf[0:HD, :], rrb)
            r65s.append(rbf)
        return r65s

    def emit_deferred_epilogue(p, ps_y2, r65s):
        for j in range(2):
            ps_y = ps_y2[j]
            if j == 0:
                nc.vector.tensor_tensor(y_sb[0:HD, p, :], ps_y[0:HD, :],
                                        r65s[j][0:HD, :], op=OP.mult)
            else:
                yt = P4.tile([128, 512], BF16, tag="tmp", name=f"yt{p}")
                nc.vector.tensor_tensor(yt[0:HD, :], ps_y[0:HD, :],
                                        r65s[j][0:HD, :], op=OP.mult)
                # partition-shift copy via the Act DMA queue: never stuck
                # behind a bulk weight transfer on the SP rings
                nc.scalar.dma_start(out=y_sb[HD:128, p, :], in_=yt[0:HD, :])

    y_sb = P1.tile([128, C6, NQ], BF16, tag="y")
    qT = P1.tile([128, C6, NQ], BF16, tag="qT")
    kch = {}
    pend = []
    exs = []

    # ---- LN1 stats / apply / Q / K0-tile0, interleaved so the in-order PE
    # queue never waits on a row chain that a later-queued matmul doesn't
    # need: stats of the next tile fill PE while the previous tile's rows,
    # broadcast and apply run on ACT/DVE/pool ----
    def emit_q(co):
        ps = ps_mm.tile([128, 512], F32, tag="mm", name=f"qps{co}")
        for ci in range(0, C6, 2):
            nc.tensor.matmul(ps, w_sb[:, ci:ci + 2, ts(co, 128)],
                             xnq[:, ci:ci + 2, :],
                             start=(ci == 0), stop=(ci == C6 - 2), perf_mode=DR)
        nc.vector.tensor_scalar(out=qT[:, co, :], in0=ps,
                                scalar1=1.0 / WS,
                                scalar2=bqkv_s[:, co:co + 1],
                                op0=OP.mult, op1=OP.add)

    # all four stats tiles first (their ACT rows stay ahead of the exp
    # stream so the Sqrt->Exp table switch happens exactly once), then the
    # minimal prefix scores tk0..3 need: apply-q, Q chunk 0, apply-k0,
    # K0 tile 0. Everything else (V, K rest, Q chunks 1..5) streams inside
    # the p-loop where PE otherwise idles.
    stq = emit_stats(xq_sb, 0, NQ, True, 0)
    stk = [None, None, None]
    stk[0] = emit_stats(xk_sb, KT[0][0], KT[0][1], False, 1)
    stk[1] = emit_stats(xk_sb, KT[1][0], KT[1][1], False, 2)
    stk[2] = emit_stats(xk_sb, KT[2][0], KT[2][1], False, 3)
    warm = P6.tile([1, 512], F32, tag="r", bufs=4, name="warm")
    nc.scalar.activation(warm[0:1, 0:8], m01[0:1, 0:8], AF.Exp)
    emit_apply(xnq, xq_sb, 0, NQ, stq[0], stq[1], 0)
    emit_q(0)
    emit_apply(xnk, xk_sb, KT[0][0], KT[0][1], stk[0][0], stk[0][1], 1)
    kch[0] = P2.tile([128, NK], BF16, tag="kch", name="kch0")
    for ci in range(0, C6, 2):
        emit_k_mm(kch[0], 0, 0, ci)
    emit_apply(xnk, xk_sb, KT[1][0], KT[1][1], stk[1][0], stk[1][1], 2)
    emit_apply(xnk, xk_sb, KT[2][0], KT[2][1], stk[2][0], stk[2][1], 3)

    if "xnq" in dbg:
        for c in range(C6):
            dt_ = P4.tile([128, 512], F32, tag="dbgt", name=f"dbxnq{c}", bufs=1)
            nc.vector.tensor_copy(out=dt_, in_=xnq[:, c, :])
            nc.sync.dma_start(out=dbg["xnq"][ts(c, 128), :], in_=dt_)
    if "xnk" in dbg:
        for c in range(C6):
            for i, (off, w) in enumerate(KT):
                dt_ = P4.tile([128, 512], F32, tag="dbgt",
                              name=f"dbxnk{c}_{i}", bufs=1)
                nc.vector.tensor_copy(out=dt_[:, 0:w],
                                      in_=xnk[:, c, off:off + w])
                nc.sync.dma_start(out=dbg["xnk"][ts(c, 128), off:off + w],
                                  in_=dt_[:, 0:w])
    if "q" in dbg:
        for c in range(C6):
            dq_ = P4.tile([128, 512], F32, tag="dbgt", name=f"dbq{c}", bufs=1)
            nc.vector.tensor_copy(out=dq_, in_=qT[:, c, :])
            nc.sync.dma_start(out=dbg["q"][ts(c, 128), :], in_=dq_)

    # ========== attention pipeline ==========
    # proj co=0 and u.y chains for p==5's spare PE slots (y chunks 0..4)
    pp0 = [None]
    ps_us = [None]

    def emit_proj_unit(which, c):
        if which == 0:
            if pp0[0] is None:
                pp0[0] = ps_mm.tile([128, 512], F32, tag="mm", name="prj0")
            nc.tensor.matmul(pp0[0], wproj_sb[:, c, ts(0, 128)], y_sb[:, c, :],
                             start=(c == 0), stop=False)
        else:
            if ps_us[0] is None:
                ps_us[0] = ps_mm.tile([1, 512], F32, tag="mm", name="ps_us")
            nc.tensor.matmul(ps_us[0], uproj_s[:, c:c + 1], y_sb[:, c, :],
                             start=(c == 0), stop=False)

    wfc2a = wfc2b = None
    for p in range(C6):
        if p == 1:
            # fc2 first half into xk's freed slot; small chunks so the
            # attention-phase bounce DMAs never queue behind a big transfer
            wfc2a = P2.tile([128, 12, D], BF16, tag="wbig", bufs=3)
            for g in range(12):
                nc.sync.dma_start(out=wfc2a[:, g, :], in_=w2r[:, g, :])
        if p == C6 - 1:
            # fc2 second half into wqkv's freed slot
            wfc2b = P2.tile([128, 12, D], BF16, tag="wbig", bufs=3)
            for g in range(12):
                nc.sync.dma_start(out=wfc2b[:, g, :], in_=w2r[:, 12 + g, :])
        if p < C6 - 1:
            kch[p + 1] = P2.tile([128, NK], BF16, tag="kch", name=f"kch{p + 1}")
            kwork = [(p + 1, ti, ci) for ti in range(3)
                     for ci in range(0, C6, 2)]
            if p == 0:
                # K0 tiles 1,2 still pending; K1 follows
                kwork = [(0, ti, ci) for ti in (1, 2)
                         for ci in range(0, C6, 2)] + kwork
        else:
            kwork = []
        vwork = list(range(K9)) if p == 0 else []
        qwork = [p + 1] if p < C6 - 1 else []
        pwork = [(w_, c) for c in range(5) for w_ in range(2)] if p == C6 - 1 \
            else []
        kper = 2 if p == 0 else 1
        ex = {}
        ps_y2 = [None, None]
        for tk in range(K9):
            pss = ps_sc.tile([128, 2, 512], F32, tag="sc")
            for j in range(2):
                po = j * 64
                nc.tensor.matmul(pss[:, j, :],
                                 kch[p][po:po + 64, ts(tk, 128)],
                                 qT[po:po + 64, p, :],
                                 start=True, stop=True)
            if tk % 2 == 0:
                ex[tk // 2] = P6.tile([128, 2, 2, 512], FP8, tag="exp",
                                      name=f"ex_{p}_{tk // 2}", bufs=5)
            nc.scalar.activation(ex[tk // 2][:, tk % 2, :, :], pss, AF.Exp)
            if vwork:
                emit_v_chunk(vwork.pop(0))
            if tk == 0 and pend:
                # previous p's last attnV chunk + softmax denominators: the
                # exp this waits on is long done, so no ACT bubble here
                emit_attnv_single(pend[-1]["p"], exs.pop(0), pend[-1]["ps_y2"])
                pend[-1]["r65s"] = emit_recips(pend[-1]["p"],
                                               pend[-1]["ps_y2"])
            if tk == 2 and pend:
                emit_deferred_epilogue(**pend.pop())
            if tk == 3:
                ps_y2[0] = ps_mm.tile([128, 512], F32, tag="mm", name=f"psyA{p}")
                ps_y2[1] = ps_mm.tile([128, 512], F32, tag="mm", name=f"psyB{p}")
                emit_attnv_pair(p, 0, ex.pop(0), ps_y2)
            if tk in (4, 6, 8):
                q = (tk - 2) // 2
                emit_attnv_pair(p, q, ex.pop(q), ps_y2)
            for _ in range(kper):
                if kwork:
                    kp, ti, ci = kwork.pop(0)
                    emit_k_mm(kch[kp], kp, ti, ci)
            if qwork and tk == 1:
                emit_q(qwork.pop())
            if pwork and tk >= 4:
                for _ in range(2):
                    w_, c = pwork.pop(0)
                    emit_proj_unit(w_, c)
        while kwork:
            kp, ti, ci = kwork.pop(0)
            emit_k_mm(kch[kp], kp, ti, ci)
        exs.append(ex.pop(4)[:, 0, :, :])
        pend.append(dict(p=p, ps_y2=ps_y2, r65s=None))
        if "k" in dbg:
            for i, (off, w) in enumerate(KT):
                dk_ = P4.tile([128, 512], F32, tag="dbgt", name=f"dbk{p}_{i}", bufs=1)
                nc.vector.tensor_copy(out=dk_[:, 0:w], in_=kch[p][:, off:off + w])
                nc.sync.dma_start(out=dbg["k"][ts(p, 128), off:off + w],
                                  in_=dk_[:, 0:w])

    # ========== proj tail + LN2 ==========
    # warm the Sqrt act table now (last exp already issued; load hides
    # under the proj matmuls)
    x2_sb = P1.tile([128, C6, NQ], F32, tag="x2")
    xn2_sb = P1.tile([128, C6, NQ], FP8, tag="xn2")
    warms = P6.tile([1, 512], F32, tag="r", bufs=4, name="warms")
    nc.scalar.activation(warms[0:1, 0:8], m01[0:1, 0:8], AF.Sqrt)
    # proj co=1,2 partial chains (need only y0..y4) fill PE while p5's
    # denominators resolve; they live in the score-psum banks, now free
    pc12 = {}
    for co in (1, 2):
        psx = ps_sc.tile([128, 512], F32, tag="sc", name=f"pcs{co}")
        for c in range(5):
            nc.tensor.matmul(psx, wproj_sb[:, c, ts(co, 128)], y_sb[:, c, :],
                             start=(c == 0), stop=False)
        pc12[co] = psx
        if co == 1:
            emit_attnv_single(pend[-1]["p"], exs.pop(0), pend[-1]["ps_y2"])
            pend[-1]["r65s"] = emit_recips(pend[-1]["p"], pend[-1]["ps_y2"])
    emit_deferred_epilogue(**pend.pop())
    if "y" in dbg:
        for c in range(C6):
            dy_ = P4.tile([128, 512], F32, tag="dbgt", name=f"dby{c}", bufs=1)
            nc.vector.tensor_copy(out=dy_, in_=y_sb[:, c, :])
            nc.sync.dma_start(out=dbg["y"][ts(c, 128), :], in_=dy_)
    nc.tensor.matmul(pp0[0], wproj_sb[:, 5, ts(0, 128)], y_sb[:, 5, :],
                     start=False, stop=True)
    nc.tensor.matmul(ps_us[0], uproj_s[:, 5:6], y_sb[:, 5, :],
                     start=False, stop=True)
    for co in (1, 2):
        nc.tensor.matmul(pc12[co], wproj_sb[:, 5, ts(co, 128)], y_sb[:, 5, :],
                         start=False, stop=True)
    nc.vector.scalar_tensor_tensor(out=x2_sb[:, 0, :], in0=pp0[0],
                                   scalar=bproj_s[:, 0:1],
                                   in1=xq_sb[:, 0, :], op0=OP.add, op1=OP.add)
    # squares on ACT (idle after the last exp; Square lives in every act
    # table set). LN2 row chain part 1 interleaves on ACT after sq0.
    sq0 = P4.tile([128, 512], BF16, tag="tmp", name="sq2_0")
    nc.scalar.activation(sq0, x2_sb[:, 0, :], AF.Square)
    srow = P6.tile([1, 512], F32, tag="r", bufs=4)
    nc.vector.scalar_tensor_tensor(out=srow, in0=ps_us[0], scalar=float(sbp),
                                   in1=sumx0, op0=OP.add, op1=OP.add)
    mrow2 = P6.tile([1, 512], BF16, tag="r", bufs=4)
    nc.scalar.activation(mrow2, srow, AF.Copy, scale=1.0 / D)
    m22 = P6.tile([1, 512], F32, tag="r", bufs=4)
    nc.scalar.activation(m22, srow, AF.Square, scale=1.0 / D)
    mbs2 = P4.tile([128, 512], BF16, tag="bcs", bufs=8)
    nc.gpsimd.partition_broadcast(mbs2, mrow2)
    d2l = [None] * C6
    sql = [sq0]
    ps_sq2 = ps_mm.tile([1, 512], F32, tag="mm")
    for co in (1, 2):
        nc.vector.scalar_tensor_tensor(out=x2_sb[:, co, :], in0=pc12[co],
                                       scalar=bproj_s[:, co:co + 1],
                                       in1=xq_sb[:, co, :],
                                       op0=OP.add, op1=OP.add)
        sq = P4.tile([128, 512], BF16, tag="tmp", name=f"sq2_{co}")
        nc.scalar.activation(sq, x2_sb[:, co, :], AF.Square)
        sql.append(sq)
    # remaining proj chunks; per-chunk: residual epilogue, square, and the
    # sumsq accumulation (behind by one, so PE never waits ACT)
    for co in range(3, C6):
        ps = ps_mm.tile([128, 512], F32, tag="mm")
        for c in range(C6):
            nc.tensor.matmul(ps, wproj_sb[:, c, ts(co, 128)], y_sb[:, c, :],
                             start=(c == 0), stop=(c == C6 - 1))
        nc.tensor.matmul(ps_sq2, ones_col, sql[co - 3],
                         start=(co == 3), stop=False)
        nc.vector.scalar_tensor_tensor(out=x2_sb[:, co, :], in0=ps,
                                       scalar=bproj_s[:, co:co + 1],
                                       in1=xq_sb[:, co, :],
                                       op0=OP.add, op1=OP.add)
        sq = P4.tile([128, 512], BF16, tag="tmp", name=f"sq2_{co}")
        nc.scalar.activation(sq, x2_sb[:, co, :], AF.Square)
        sql.append(sq)
        c = co - 3
        d2l[c] = P4.tile([128, 512], BF16, tag="bcs", name=f"d2_{c}", bufs=8)
        nc.vector.tensor_tensor(d2l[c], x2_sb[:, c, :], mbs2, op=OP.subtract)
    for c in range(3, C6):
        nc.tensor.matmul(ps_sq2, ones_col, sql[c],
                         start=False, stop=(c == C6 - 1))

    if "x2" in dbg:
        for c in range(C6):
            dx2_ = P4.tile([128, 512], F32, tag="dbgt", name=f"dbx2{c}", bufs=1)
            nc.vector.tensor_copy(out=dx2_, in_=x2_sb[:, c, :])
            nc.sync.dma_start(out=dbg["x2"][ts(c, 128), :], in_=dx2_)
    # ========== LN2 rest of row chain ==========
    vrow2 = P6.tile([1, 512], F32, tag="r", bufs=4)
    nc.vector.scalar_tensor_tensor(out=vrow2, in0=ps_sq2, scalar=1.0 / D,
                                   in1=m22, op0=OP.mult, op1=OP.subtract)
    srt2 = P6.tile([1, 512], F32, tag="r", bufs=4)
    nc.scalar.activation(srt2, vrow2, AF.Sqrt, bias=eps1)
    warm2 = P6.tile([1, 512], F32, tag="r", bufs=4, name="warm2")
    nc.scalar.activation(warm2[0:1, 0:8], srt2[0:1, 0:8], AF.Gelu)
    rf2 = P6.tile([1, 512], F32, tag="r", bufs=4)
    nc.vector.reciprocal_approx_fast(out=rf2, in_=srt2)
    rrow2 = P6.tile([1, 512], BF16, tag="r", bufs=4)
    nc.vector.tensor_copy(out=rrow2, in_=rf2)
    for c in (3, 4, 5):
        d2l[c] = P4.tile([128, 512], BF16, tag="bcs", name=f"d2_{c}", bufs=8)
        nc.vector.tensor_tensor(d2l[c], x2_sb[:, c, :], mbs2, op=OP.subtract)
    rbs2 = P4.tile([128, 512], BF16, tag="bcs", bufs=8)
    nc.gpsimd.partition_broadcast(rbs2, rrow2)
    for c in range(C6):
        nc.vector.tensor_tensor(xn2_sb[:, c, :], d2l[c], rbs2, op=OP.mult)

    # ========== MLP (fc1 fp8 DoubleRow, fc2 bf16) ==========
    h_sb = P1.tile([128, HO24, NQ], BF16, tag="h")
    for ho in range(HO24):
        ps = ps_mm.tile([128, 512], F32, tag="mm")
        for ci in range(0, C6, 2):
            nc.tensor.matmul(ps, wfc1_sb[:, ci:ci + 2, ts(ho, 128)],
                             xn2_sb[:, ci:ci + 2, :],
                             start=(ci == 0), stop=(ci == C6 - 2), perf_mode=DR)
        nc.scalar.activation(h_sb[:, ho, :], ps, AF.Gelu,
                             scale=1.0 / WS, bias=bfc1_s[:, ho:ho + 1])
    for co in range(C6):
        ps = ps_mm.tile([128, 512], F32, tag="mm")
        for ho in range(HO24):
            wsrc = wfc2a if ho < 12 else wfc2b
            nc.tensor.matmul(ps, wsrc[:, ho % 12, ts(co, 128)], h_sb[:, ho, :],
                             start=(ho == 0), stop=(ho == HO24 - 1))
        o = P2.tile([128, 512], F32, tag="ot", name=f"o_{co}")
        nc.vector.scalar_tensor_tensor(out=o, in0=ps,
                                       scalar=bfc2_s[:, co:co + 1],
                                       in1=x2_sb[:, co, :],
                                       op0=OP.add, op1=OP.add)
        nc.sync.dma_start(out=out_d[ts(co, 128), :], in_=o)

    for cm in (dr_cm, ps_sc_cm, ps_mm_cm, P6_cm, P4_cm, P2_cm, P1_cm):
        cm.__exit__(None, None, None)


def _host_prep(x, mask, ln1_g, ln1_b, qkv_w, proj_w, proj_b, ln2_g, ln2_b,
               fc1_w, fc1_b, fc2_w, fc2_b):
    bf = ml_dtypes.bfloat16
    f8 = ml_dtypes.float8_e4m3
    f32 = np.float32
    x = np.asarray(x, f32)
    mask = np.asarray(mask)
    qkv_w = np.asarray(qkv_w, f32)
    proj_w = np.asarray(proj_w, f32)
    fc1_w = np.asarray(fc1_w, f32)
    fc2_w = np.asarray(fc2_w, f32)
    ln1_g = np.asarray(ln1_g, f32); ln1_b = np.asarray(ln1_b, f32)
    ln2_g = np.asarray(ln2_g, f32); ln2_b = np.asarray(ln2_b, f32)
    proj_b = np.asarray(proj_b, f32)
    fc1_b = np.asarray(fc1_b, f32); fc2_b = np.asarray(fc2_b, f32)

    wqkv_f = qkv_w * ln1_g[None, :]
    bqkv_f = qkv_w @ ln1_b
    wqkv_f[0:D] *= SCALE
    bqkv_f[0:D] *= SCALE
    bv = bqkv_f[2 * D:3 * D].copy()
    bqkv_f[2 * D:3 * D] = 0.0     # v bias folded into proj bias (sum(attn)=1)
    bproj_f = proj_b + proj_w @ bv
    wfc1_f = fc1_w * ln2_g[None, :]
    bfc1_f = fc1_w @ ln2_b + fc1_b

    shared = {
        "wqkv": np.ascontiguousarray(wqkv_f.T * WS).astype(f8),
        "wproj": np.ascontiguousarray(proj_w.T).astype(bf),
        "wfc1": np.ascontiguousarray(wfc1_f.T * WS).astype(f8),
        "wfc2": np.ascontiguousarray(fc2_w.T).astype(bf),
        "bqkv": np.ascontiguousarray(bqkv_f.reshape(18, 128).T).astype(f32),
        "bproj": np.ascontiguousarray(bproj_f.reshape(6, 128).T).astype(f32),
        "bfc1": np.ascontiguousarray(bfc1_f.reshape(24, 128).T).astype(f32),
        "bfc2": np.ascontiguousarray(fc2_b.reshape(6, 128).T).astype(f32),
        "uproj": np.ascontiguousarray(
            proj_w.sum(axis=0).reshape(6, 128).T).astype(bf),
    }
    sbp = float(bproj_f.sum())

    # per-batch compacted key set (host-side gather of unmasked tokens)
    xk_b, m01_b = [], []
    for b in range(B):
        idx = np.nonzero(mask[b] != 1)[0]
        nk = len(idx)
        assert nk <= NK, f"batch {b}: {nk} unmasked keys > NK={NK}"
        xk = np.zeros((NK, D), f32)
        xk[:nk] = x[b][idx]
        m01 = np.zeros((NK,), f32)
        m01[:nk] = WS   # cancels the WS carried by the V weights
        xk_b.append(np.ascontiguousarray(xk.T).astype(bf))
        m01_b.append(np.ascontiguousarray(m01.reshape(K9, 128).T).astype(f32))

    in_maps = []
    for core in range(NC):
        b, s = divmod(core, NSH)
        im = dict(shared)
        im["xqT"] = np.ascontiguousarray(
            x[b, s * NQ:(s + 1) * NQ].T).astype(bf)
        im["xkT"] = xk_b[b]
        im["mask01"] = m01_b[b]
        in_maps.append(im)
    return in_maps, sbp


def kernel(**inputs):
    in_maps, sbp = _host_prep(**inputs)
    if _cached.get("sbp") != sbp:
        _cached["nc"] = _build_nc(sbp)
        _cached["sbp"] = sbp
    res = run_bass_kernel_spmd(_cached["nc"], in_maps, core_ids=list(range(NC)))
    out = np.empty((B, N, D), np.float32)
    for core in range(NC):
        b, s = divmod(core, NSH)
        out[b, s * NQ:(s + 1) * NQ, :] = res.results[core]["out"].T
    return out


# revision 63
# speedup vs baseline: 1.0211x; 1.0036x over previous
"""Trainium2 Bass kernel for a dense transformer block (pre-LN, MHA + GELU MLP).

Problem shapes (hardcoded): x [2, 2048, 768] f32, mask [2, 2048] int32,
12 heads x 64 dims, hidden 3072.

Sharding: 8 cores = (batch b in {0,1}) x (query shard s in {0..3}).
Each core gets its 512-query slice of x (xqT) plus the HOST-COMPACTED set
of unmasked key tokens of its batch (xkT, padded to NK=1152). Masked keys
(~half of all tokens) never reach the device: K/V projection, scores, exp
and attnV all shrink by ~44% vs processing all 2048 keys. Padding keys are
zero (=> v rows 0) and their softmax-denominator entry is zeroed via m01.

On-chip layout is feature-major: activations are [features, tokens], every
matmul contracts over the partition dim. LN gain/bias are folded into the
next matmul's weights host-side; per-token mean/rstd come from ones-vector
matmuls (partition reduction on PE) and are broadcast back across
partitions with gpsimd partition_broadcast.

fp8 (e4m3, x32 weight scale) with DoubleRow is used for qkv, fc1 and fc2
matmuls. Softmax denominators come free from a WS-scaled ones column
appended to V (M=65 attnV matmul); the WS of the V weights cancels against
it so the V epilogue is a pure f32->fp8 cast. Max-subtraction is skipped:
|scores| <= ~4 by construction, so exp cannot overflow.
"""

import numpy as np
import ml_dtypes

import concourse.bass as bass
import concourse.tile as tile
import concourse.mybir as mybir
from concourse import bacc
from concourse.bass import ts
from concourse.bass_utils import run_bass_kernel_spmd
from concourse.alu_op_type import AluOpType

BF16 = mybir.dt.bfloat16
F32 = mybir.dt.float32
FP8 = mybir.dt.float8e4
DR = mybir.MatmulPerfMode.DoubleRow
WS = 32.0   # fp8 weight scale (dodges e4m3 subnormals)

B = 2
N = 2048
D = 768
H = 12
HD = 64
HID = 3072
EPS = 1e-5
SCALE = HD ** -0.5
NQ = 512          # queries per core
NSH = N // NQ     # query shards per batch
NC = B * NSH      # 8 cores
C6 = D // 128     # feature chunks
NK = 1152         # padded compacted key count per batch
K9 = NK // 128    # key chunks
HO24 = HID // 128
# key tiles for LN stats / apply / K-proj free dim (offset, width)
KT = [(0, 512), (512, 512), (1024, 128)]

AF = mybir.ActivationFunctionType
OP = AluOpType

_cached = {}
_rid = [0]


def _build_nc(sbp):
    nc = bacc.Bacc("TRN2", target_bir_lowering=False, debug=False,
                   enable_asserts=False, num_devices=NC)

    xqT = nc.dram_tensor("xqT", [D, NQ], BF16, kind="ExternalInput").ap()
    xkT = nc.dram_tensor("xkT", [D, NK], BF16, kind="ExternalInput").ap()
    wqkv = nc.dram_tensor("wqkv", [D, 3 * D], FP8, kind="ExternalInput").ap()
    wproj = nc.dram_tensor("wproj", [D, D], BF16, kind="ExternalInput").ap()
    wfc1 = nc.dram_tensor("wfc1", [D, HID], FP8, kind="ExternalInput").ap()
    wfc2 = nc.dram_tensor("wfc2", [HID, D], BF16, kind="ExternalInput").ap()
    bqkv = nc.dram_tensor("bqkv", [128, 18], F32, kind="ExternalInput").ap()
    bproj = nc.dram_tensor("bproj", [128, 6], F32, kind="ExternalInput").ap()
    bfc1 = nc.dram_tensor("bfc1", [128, 24], F32, kind="ExternalInput").ap()
    bfc2 = nc.dram_tensor("bfc2", [128, 6], F32, kind="ExternalInput").ap()
    uproj = nc.dram_tensor("uproj", [128, 6], BF16, kind="ExternalInput").ap()
    mask01 = nc.dram_tensor("mask01", [128, K9], F32, kind="ExternalInput").ap()
    out_d = nc.dram_tensor("out", [D, NQ], F32, kind="ExternalOutput").ap()
    import os
    dbg = {}
    if os.environ.get("KDBG"):
        dbg["y"] = nc.dram_tensor("dbg_y", [D, NQ], F32, kind="ExternalOutput").ap()
        dbg["x2"] = nc.dram_tensor("dbg_x2", [D, NQ], F32, kind="ExternalOutput").ap()
        dbg["xnq"] = nc.dram_tensor("dbg_xnq", [D, NQ], F32, kind="ExternalOutput").ap()
        dbg["xnk"] = nc.dram_tensor("dbg_xnk", [D, NK], F32, kind="ExternalOutput").ap()
        dbg["q"] = nc.dram_tensor("dbg_q", [D, NQ], F32, kind="ExternalOutput").ap()
        dbg["k"] = nc.dram_tensor("dbg_k", [D, NK], F32, kind="ExternalOutput").ap()
        dbg["yu"] = nc.dram_tensor("dbg_yu", [H, HD + 1, NQ], F32, kind="ExternalOutput").ap()

    with tile.TileContext(nc) as tc:
        _body(nc, tc, sbp, xqT, xkT, wqkv, wproj, wfc1, wfc2, bqkv, bproj,
              bfc1, bfc2, uproj, mask01, out_d, dbg)
    nc.compile()
    return nc


def _body(nc, tc, sbp, xqT, xkT, wqkv_d, wproj_d, wfc1_d, wfc2_d, bqkv_d,
          bproj_d, bfc1_d, bfc2_d, uproj_d, mask01_d, out_d, dbg=None):
    dbg = dbg or {}
    P1_cm = tc.tile_pool(name="p1", bufs=1); P1 = P1_cm.__enter__()
    P2_cm = tc.tile_pool(name="p2", bufs=2); P2 = P2_cm.__enter__()
    P4_cm = tc.tile_pool(name="p4", bufs=4); P4 = P4_cm.__enter__()
    P6_cm = tc.tile_pool(name="p6", bufs=6); P6 = P6_cm.__enter__()
    ps_mm_cm = tc.tile_pool(name="ps_mm", bufs=4, space="PSUM")
    ps_mm = ps_mm_cm.__enter__()
    ps_sc_cm = tc.tile_pool(name="ps_sc", bufs=2, space="PSUM")
    ps_sc = ps_sc_cm.__enter__()
    dr_cm = tc.tile_pool(name="drp", bufs=8, space="DRAM")
    drp = dr_cm.__enter__()

    def bcast_dma(dst, src_row):
        """replicate a single-partition row across dst partitions:
        SBUF row -> DRAM scratch -> partition-broadcast DMA back"""
        dt = drp.tile([1, 512], src_row.dtype, tag="dr",
                      name=f"dr{_rid[0]}")[:, 0:src_row.free_size()]
        _rid[0] += 1
        nc.sync.dma_start(out=dt, in_=src_row)
        rap = bass.AP(tensor=dt.tensor, offset=dt.offset,
                      ap=[[0, dst.shape[0]]] + [list(x) for x in dt.ap[1:]])
        nc.sync.dma_start(out=dst, in_=rap)

    # ---- constants ----
    ones = P1.tile([128, 128], BF16, tag="ones")
    nc.vector.memset(ones, 1.0)
    ones_col = ones[:, 0:1]
    m01 = P1.tile([128, K9], F32, tag="m01")
    nc.sync.dma_start(out=m01, in_=mask01_d)
    bqkv_s = P1.tile([128, 18], F32, tag="bqkv")
    nc.sync.dma_start(out=bqkv_s, in_=bqkv_d)
    bproj_s = P1.tile([128, 6], F32, tag="bproj")
    nc.sync.dma_start(out=bproj_s, in_=bproj_d)
    bfc1_s = P1.tile([128, 24], F32, tag="bfc1")
    nc.sync.dma_start(out=bfc1_s, in_=bfc1_d)
    bfc2_s = P1.tile([128, 6], F32, tag="bfc2")
    nc.sync.dma_start(out=bfc2_s, in_=bfc2_d)
    uproj_s = P1.tile([128, 6], BF16, tag="uproj")
    nc.sync.dma_start(out=uproj_s, in_=uproj_d)
    eps1 = P1.tile([1, 1], F32, tag="eps1")
    nc.vector.memset(eps1, EPS)
    sumx0 = P1.tile([1, NQ], BF16, tag="sumx0")

    # ---- activations / weights to SBUF ----
    # xk / wqkv / wfc1 / wfc2a / wfc2b have disjoint lifetimes: rotate them
    # through 3 pool slots (xk dies after LN1 apply -> wfc2a takes its slot;
    # wqkv dies after the last K projection -> wfc2b takes its slot)
    xq_sb = P1.tile([128, C6, NQ], BF16, tag="xq")
    xqr = xqT.rearrange("(a p) q -> p a q", p=128)
    for c in range(C6):
        for hf in range(2):
            nc.sync.dma_start(out=xq_sb[:, c, ts(hf, NQ // 2)],
                              in_=xqr[:, c, ts(hf, NQ // 2)])
    xk_sb = P2.tile([128, C6, NK], BF16, tag="wbig", bufs=3)
    xkr = xkT.rearrange("(a p) q -> p a q", p=128)
    for c in range(C6):
        for hf in range(2):
            nc.sync.dma_start(out=xk_sb[:, c, ts(hf, NK // 2)],
                              in_=xkr[:, c, ts(hf, NK // 2)])
    w_sb = P2.tile([128, C6, 3 * D], FP8, tag="wbig", bufs=3)
    wqr = wqkv_d.rearrange("(a p) o -> p a o", p=128)
    for c in range(C6):
        nc.sync.dma_start(out=w_sb[:, c, :], in_=wqr[:, c, :])
    wfc1_sb = P2.tile([128, C6, HID], FP8, tag="wbig", bufs=3)
    w1r = wfc1_d.rearrange("(a p) o -> p a o", p=128)
    for c in range(C6):
        nc.sync.dma_start(out=wfc1_sb[:, c, :], in_=w1r[:, c, :])
    wproj_sb = P1.tile([128, C6, D], BF16, tag="wproj")
    wpr = wproj_d.rearrange("(a p) o -> p a o", p=128)
    for c in range(C6):
        nc.sync.dma_start(out=wproj_sb[:, c, :], in_=wpr[:, c, :])
    w2r = wfc2_d.rearrange("(a p) o -> p a o", p=128)

    xnq = P1.tile([128, C6, NQ], FP8, tag="xnq")
    xnk = P1.tile([128, C6, NK], FP8, tag="xnk")

    # ========== LN1 ==========
    def emit_stats_pe(src3, off, w, is_q, t):
        ps_sum = ps_mm.tile([1, 512], F32, tag="mm", name=f"pssum{t}")
        ps_sq = ps_mm.tile([1, 512], F32, tag="mm", name=f"pssq{t}")
        for c in range(C6):
            xa = src3[:, c, off:off + w]
            sq = P4.tile([128, 512], BF16, tag="tmp", name=f"sq{t}_{c}")
            nc.vector.tensor_tensor(sq[:, 0:w], xa, xa, op=OP.mult)
            nc.tensor.matmul(ps_sum[:, 0:w], ones_col, xa,
                             start=(c == 0), stop=(c == C6 - 1))
            nc.tensor.matmul(ps_sq[:, 0:w], ones_col, sq[:, 0:w],
                             start=(c == 0), stop=(c == C6 - 1))
        if is_q:
            nc.vector.tensor_copy(out=sumx0, in_=ps_sum[:, 0:w])
        return ps_sum, ps_sq

    def emit_stats_rows(srcs, w, t):
        ps_sum, ps_sq = srcs
        mrow = P6.tile([1, 512], BF16, tag="mrow", name=f"mrow{t}", bufs=2)
        nc.scalar.activation(mrow[:, 0:w], ps_sum[:, 0:w], AF.Copy, scale=1.0 / D)
        m2 = P6.tile([1, 512], F32, tag="r", name=f"m2_{t}", bufs=4)
        nc.scalar.activation(m2[:, 0:w], ps_sum[:, 0:w], AF.Square, scale=1.0 / D)
        vrow = P6.tile([1, 512], F32, tag="vrow", name=f"vrow{t}", bufs=2)
        nc.vector.scalar_tensor_tensor(out=vrow[:, 0:w], in0=ps_sq[:, 0:w],
                                       scalar=1.0 / D, in1=m2[:, 0:w],
                                       op0=OP.mult, op1=OP.subtract)
        srt = P6.tile([1, 512], F32, tag="r", name=f"srt{t}", bufs=4)
        nc.scalar.activation(srt[:, 0:w], vrow[:, 0:w], AF.Sqrt, bias=eps1)
        rf = P6.tile([1, 512], F32, tag="r", name=f"rf{t}", bufs=4)
        nc.vector.reciprocal_approx_fast(out=rf[:, 0:w], in_=srt[:, 0:w])
        rrow = P6.tile([1, 512], BF16, tag="rrow", name=f"rrow{t}", bufs=2)
        nc.vector.tensor_copy(out=rrow[:, 0:w], in_=rf[:, 0:w])
        mbs = P4.tile([128, 512], BF16, tag="bcs", name=f"mbs{t}", bufs=8)
        nc.gpsimd.partition_broadcast(mbs[:, 0:w], mrow[:, 0:w])
        rbs_ = P4.tile([128, 512], BF16, tag="bcs", name=f"rbs_{t}", bufs=8)
        nc.gpsimd.partition_broadcast(rbs_[:, 0:w], rrow[:, 0:w])
        return mbs, rbs_

    def emit_apply(dst, src3, off, w, mbs, rbs_, t):
        for c in range(C6):
            d = P4.tile([128, 512], BF16, tag="tmp", name=f"d{t}_{c}")
            nc.vector.tensor_tensor(d[:, 0:w], src3[:, c, off:off + w],
                                    mbs[:, 0:w], op=OP.subtract)
            nc.vector.tensor_tensor(dst[:, c, off:off + w], d[:, 0:w],
                                    rbs_[:, 0:w], op=OP.mult)

    # ---- attention data structures / helpers ----
    # vsb per key chunk: 12 heads x (64 v-dims + 1 denom col), padded to 784
    vsb = P1.tile([128, K9, 16 * ((H * (HD + 1) + 15) // 16)], FP8, tag="vsb")
    m01r = bass.AP(tensor=m01.tensor, offset=m01.offset,
                   ap=[list(m01.ap[0]), list(m01.ap[1]), [0, H], [0, 1]])
    vsb_h = vsb[:, :, 0:H * (HD + 1)].rearrange("p k (h e) -> p k h e", e=HD + 1)
    nc.vector.tensor_copy(out=vsb_h[:, :, :, HD:HD + 1], in_=m01r)

    kch_state = [None]

    def emit_k_mm(kch_p, p, ti, ci):
        off, w = KT[ti]
        if ci == 0:
            kch_state[0] = ps_mm.tile([128, 512], F32, tag="mm",
                                      name=f"kst{_rid[0]}")
            _rid[0] += 1
        nc.tensor.matmul(kch_state[0][:, 0:w], w_sb[:, ci:ci + 2, ts(6 + p, 128)],
                         xnk[:, ci:ci + 2, off:off + w],
                         start=(ci == 0), stop=(ci == C6 - 2), perf_mode=DR)
        if ci == C6 - 2:
            nc.vector.tensor_scalar(out=kch_p[:, off:off + w],
                                    in0=kch_state[0][:, 0:w],
                                    scalar1=1.0 / WS,
                                    scalar2=bqkv_s[:, 6 + p:6 + p + 1],
                                    op0=OP.mult, op1=OP.add)
            kch_state[0] = None

    def emit_v_chunk(tk):
        # psum keeps the WS weight scale; it cancels against the WS-scaled
        # denominator column, so this epilogue is a pure f32->fp8 cast
        for half in range(2):
            psv = ps_mm.tile([128, 512], F32, tag="mm", name=f"psv{tk}_{half}")
            for ci in range(0, C6, 2):
                nc.tensor.matmul(psv[:, 0:384],
                                 xnk[:, ci:ci + 2, ts(tk, 128)],
                                 w_sb[:, ci:ci + 2, 12 * 128 + half * 384:
                                      12 * 128 + (half + 1) * 384],
                                 start=(ci == 0), stop=(ci == C6 - 2),
                                 perf_mode=DR)
            vout = vsb[:, tk, half * 390:half * 390 + 390].rearrange(
                "p (h e) -> p h e", e=HD + 1)[:, :, 0:HD]
            nc.vector.tensor_copy(
                out=vout, in_=psv[:, 0:384].rearrange("p (h d) -> p h d", h=6))

    def emit_attnv_pair(p, q, ex2t, ps_y2):
        for j in range(2):
            h = 2 * p + j
            nc.tensor.matmul(ps_y2[j][0:HD + 1, :],
                             vsb[:, 2 * q:2 * q + 2, h * 65:h * 65 + 65],
                             ex2t[:, :, j, :],
                             start=(q == 0), stop=False, perf_mode=DR)

    def emit_attnv_single(p, ex1, ps_y2):
        for j in range(2):
            h = 2 * p + j
            nc.tensor.matmul(ps_y2[j][0:HD + 1, :],
                             vsb[:, K9 - 1, h * 65:h * 65 + 65],
                             ex1[:, j, :],
                             start=False, stop=True)

    def emit_recips(p, ps_y2):
        r65s = []
        for j in range(2):
            if "yu" in dbg:
                du_ = P4.tile([128, 512], F32, tag="dbgt", name=f"dyu{p}_{j}", bufs=1)
                nc.vector.tensor_copy(out=du_[0:HD + 1, :],
                                      in_=ps_y2[j][0:HD + 1, :])
                nc.sync.dma_start(out=dbg["yu"][2 * p + j, :, :],
                                  in_=du_[0:HD + 1, :])
            sr = P6.tile([1, 512], F32, tag="rr", name=f"sr{p}_{j}", bufs=2)
            nc.vector.tensor_copy(out=sr, in_=ps_y2[j][HD:HD + 1, :])
            rr = P6.tile([1, 512], F32, tag="rr", name=f"rr{p}_{j}", bufs=2)
            nc.vector.reciprocal_approx_fast(out=rr, in_=sr)
            rrb = P6.tile([1, 512], BF16, tag="rrb", name=f"rrb{p}_{j}", bufs=2)
            nc.vector.tensor_copy(out=rrb, in_=rr)
            rbf = P4.tile([128, 512], BF16, tag="tf", name=f"rbf{p}_{j}")
            nc.gpsimd.partition_broadcast(rbf[0:HD, :], rrb)
            r65s.append(rbf)
        return r65s

    def emit_deferred_epilogue(p, ps_y2, r65s, last=False):
        for j in range(2):
            ps_y = ps_y2[j]
            if j == 0:
                nc.vector.tensor_tensor(y_sb[0:HD, p, :], ps_y[0:HD, :],
                                        r65s[j][0:HD, :], op=OP.mult)
            else:
                yt = P4.tile([128, 512], BF16, tag="tmp", name=f"yt{p}")
                nc.vector.tensor_tensor(yt[0:HD, :], ps_y[0:HD, :],
                                        r65s[j][0:HD, :], op=OP.mult)
                # partition-shift copy via the Act DMA queue: never stuck
                # behind a bulk weight transfer on the SP rings. For the
                # last pair use the (now quiet) SP rings instead, so this
                # wait doesn't head-of-line block the LN2 squares on ACT
                eng = nc.sync if last else nc.scalar
                eng.dma_start(out=y_sb[HD:128, p, :], in_=yt[0:HD, :])

    y_sb = P1.tile([128, C6, NQ], BF16, tag="y")
    qT = P1.tile([128, C6, NQ], BF16, tag="qT")
    kch = {}
    pend = []
    exs = []

    # ---- LN1 stats / apply / Q / K0-tile0, interleaved so the in-order PE
    # queue never waits on a row chain that a later-queued matmul doesn't
    # need: stats of the next tile fill PE while the previous tile's rows,
    # broadcast and apply run on ACT/DVE/pool ----
    def emit_q(co):
        ps = ps_mm.tile([128, 512], F32, tag="mm", name=f"qps{co}")
        for ci in range(0, C6, 2):
            nc.tensor.matmul(ps, w_sb[:, ci:ci + 2, ts(co, 128)],
                             xnq[:, ci:ci + 2, :],
                             start=(ci == 0), stop=(ci == C6 - 2), perf_mode=DR)
        nc.vector.tensor_scalar(out=qT[:, co, :], in0=ps,
                                scalar1=1.0 / WS,
                                scalar2=bqkv_s[:, co:co + 1],
                                op0=OP.mult, op1=OP.add)

    # all four stats tiles first (their ACT rows stay ahead of the exp
    # stream so the Sqrt->Exp table switch happens exactly once), then the
    # minimal prefix scores tk0..3 need: apply-q, Q chunk 0, apply-k0,
    # K0 tile 0. Everything else (V, K rest, Q chunks 1..5) streams inside
    # the p-loop where PE otherwise idles.
    # one-tile-lag pipeline: tile t's DVE row ops (which wait on t's PE
    # sums) are emitted after tile t+1's square-feeds, so the in-order DVE
    # queue never blocks the next tile's PE stats. Rows stay only one tile
    # behind, so no stats PSUM pair outlives the next pair's allocation.
    psq_t = emit_stats_pe(xq_sb, 0, NQ, True, 0)
    psk0 = emit_stats_pe(xk_sb, KT[0][0], KT[0][1], False, 1)
    stq = emit_stats_rows(psq_t, NQ, 0)
    psk1 = emit_stats_pe(xk_sb, KT[1][0], KT[1][1], False, 2)
    stk = [None, None, None]
    stk[0] = emit_stats_rows(psk0, KT[0][1], 1)
    psk2 = emit_stats_pe(xk_sb, KT[2][0], KT[2][1], False, 3)
    stk[1] = emit_stats_rows(psk1, KT[1][1], 2)
    stk[2] = emit_stats_rows(psk2, KT[2][1], 3)
    warm = P6.tile([1, 512], F32, tag="r", bufs=4, name="warm")
    nc.scalar.activation(warm[0:1, 0:8], m01[0:1, 0:8], AF.Exp)
    emit_apply(xnq, xq_sb, 0, NQ, stq[0], stq[1], 0)
    emit_q(0)
    emit_apply(xnk, xk_sb, KT[0][0], KT[0][1], stk[0][0], stk[0][1], 1)
    kch[0] = P2.tile([128, NK], BF16, tag="kch", name="kch0")
    for ci in range(0, C6, 2):
        emit_k_mm(kch[0], 0, 0, ci)
    emit_apply(xnk, xk_sb, KT[1][0], KT[1][1], stk[1][0], stk[1][1], 2)
    emit_apply(xnk, xk_sb, KT[2][0], KT[2][1], stk[2][0], stk[2][1], 3)

    if "xnq" in dbg:
        for c in range(C6):
            dt_ = P4.tile([128, 512], F32, tag="dbgt", name=f"dbxnq{c}", bufs=1)
            nc.vector.tensor_copy(out=dt_, in_=xnq[:, c, :])
            nc.sync.dma_start(out=dbg["xnq"][ts(c, 128), :], in_=dt_)
    if "xnk" in dbg:
        for c in range(C6):
            for i, (off, w) in enumerate(KT):
                dt_ = P4.tile([128, 512], F32, tag="dbgt",
                              name=f"dbxnk{c}_{i}", bufs=1)
                nc.vector.tensor_copy(out=dt_[:, 0:w],
                                      in_=xnk[:, c, off:off + w])
                nc.sync.dma_start(out=dbg["xnk"][ts(c, 128), off:off + w],
                                  in_=dt_[:, 0:w])
    if "q" in dbg:
        for c in range(C6):
            dq_ = P4.tile([128, 512], F32, tag="dbgt", name=f"dbq{c}", bufs=1)
            nc.vector.tensor_copy(out=dq_, in_=qT[:, c, :])
            nc.sync.dma_start(out=dbg["q"][ts(c, 128), :], in_=dq_)

    # ========== attention pipeline ==========
    # proj co=0 and u.y chains for p==5's spare PE slots (y chunks 0..4)
    pp0 = [None]
    ps_us = [None]

    def emit_proj_unit(which, c):
        if which == 0:
            if pp0[0] is None:
                pp0[0] = ps_mm.tile([128, 512], F32, tag="mm", name="prj0")
            nc.tensor.matmul(pp0[0], wproj_sb[:, c, ts(0, 128)], y_sb[:, c, :],
                             start=(c == 0), stop=False)
        else:
            if ps_us[0] is None:
                ps_us[0] = ps_mm.tile([1, 512], F32, tag="mm", name="ps_us")
            nc.tensor.matmul(ps_us[0], uproj_s[:, c:c + 1], y_sb[:, c, :],
                             start=(c == 0), stop=False)

    wfc2a = wfc2b = None
    for p in range(C6):
        if p == 1:
            # fc2 first half into xk's freed slot; small chunks so the
            # attention-phase bounce DMAs never queue behind a big transfer
            wfc2a = P2.tile([128, 12, D], BF16, tag="wbig", bufs=3)
            for g in range(12):
                nc.sync.dma_start(out=wfc2a[:, g, :], in_=w2r[:, g, :])
        if p == C6 - 1:
            # fc2 second half into wqkv's freed slot
            wfc2b = P2.tile([128, 12, D], BF16, tag="wbig", bufs=3)
            for g in range(12):
                nc.sync.dma_start(out=wfc2b[:, g, :], in_=w2r[:, 12 + g, :])
        if p < C6 - 1:
            kch[p + 1] = P2.tile([128, NK], BF16, tag="kch", name=f"kch{p + 1}")
            kwork = [(p + 1, ti, ci) for ti in range(3)
                     for ci in range(0, C6, 2)]
            if p == 0:
                # K0 tiles 1,2 still pending; K1 follows
                kwork = [(0, ti, ci) for ti in (1, 2)
                         for ci in range(0, C6, 2)] + kwork
        else:
            kwork = []
        vwork = list(range(K9)) if p == 0 else []
        qwork = [p + 1] if p < C6 - 1 else []
        pwork = [(w_, c) for c in range(5) for w_ in range(2)] if p == C6 - 1 \
            else []
        kper = 2 if p == 0 else 1
        ex = {}
        ps_y2 = [None, None]
        for tk in range(K9):
            pss = ps_sc.tile([128, 2, 512], F32, tag="sc")
            for j in range(2):
                po = j * 64
                nc.tensor.matmul(pss[:, j, :],
                                 kch[p][po:po + 64, ts(tk, 128)],
                                 qT[po:po + 64, p, :],
                                 start=True, stop=True)
            if tk % 2 == 0:
                ex[tk // 2] = P6.tile([128, 2, 2, 512], FP8, tag="exp",
                                      name=f"ex_{p}_{tk // 2}", bufs=5)
            nc.scalar.activation(ex[tk // 2][:, tk % 2, :, :], pss, AF.Exp)
            if vwork:
                emit_v_chunk(vwork.pop(0))
            if tk == 0 and pend:
                # previous p's last attnV chunk + softmax denominators: the
                # exp this waits on is long done, so no ACT bubble here
                emit_attnv_single(pend[-1]["p"], exs.pop(0), pend[-1]["ps_y2"])
                pend[-1]["r65s"] = emit_recips(pend[-1]["p"],
                                               pend[-1]["ps_y2"])
            if tk == 2 and pend:
                emit_deferred_epilogue(**pend.pop())
            if tk == 3:
                ps_y2[0] = ps_mm.tile([128, 512], F32, tag="mm", name=f"psyA{p}")
                ps_y2[1] = ps_mm.tile([128, 512], F32, tag="mm", name=f"psyB{p}")
                emit_attnv_pair(p, 0, ex.pop(0), ps_y2)
            if tk in (4, 6, 8):
                q = (tk - 2) // 2
                emit_attnv_pair(p, q, ex.pop(q), ps_y2)
            for _ in range(kper):
                if kwork:
                    kp, ti, ci = kwork.pop(0)
                    emit_k_mm(kch[kp], kp, ti, ci)
            if qwork and tk == 1:
                emit_q(qwork.pop())
            if pwork and tk >= 4:
                for _ in range(2):
                    w_, c = pwork.pop(0)
                    emit_proj_unit(w_, c)
        while kwork:
            kp, ti, ci = kwork.pop(0)
            emit_k_mm(kch[kp], kp, ti, ci)
        exs.append(ex.pop(4)[:, 0, :, :])
        pend.append(dict(p=p, ps_y2=ps_y2, r65s=None))
        if "k" in dbg:
            for i, (off, w) in enumerate(KT):
                dk_ = P4.tile([128, 512], F32, tag="dbgt", name=f"dbk{p}_{i}", bufs=1)
                nc.vector.tensor_copy(out=dk_[:, 0:w], in_=kch[p][:, off:off + w])
                nc.sync.dma_start(out=dbg["k"][ts(p, 128), off:off + w],
                                  in_=dk_[:, 0:w])

    # ========== proj tail + LN2 ==========
    # warm the Sqrt act table now (last exp already issued; load hides
    # under the proj matmuls)
    x2_sb = P1.tile([128, C6, NQ], F32, tag="x2")
    xn2_sb = P1.tile([128, C6, NQ], FP8, tag="xn2")
    warms = P6.tile([1, 512], F32, tag="r", bufs=4, name="warms")
    nc.scalar.activation(warms[0:1, 0:8], m01[0:1, 0:8], AF.Sqrt)
    # proj co=1,2 partial chains (need only y0..y4) fill PE while p5's
    # denominators resolve; they live in the score-psum banks, now free
    pc12 = {}
    for co in (1, 2):
        psx = ps_sc.tile([128, 512], F32, tag="sc", name=f"pcs{co}")
        for c in range(5):
            nc.tensor.matmul(psx, wproj_sb[:, c, ts(co, 128)], y_sb[:, c, :],
                             start=(c == 0), stop=False)
        pc12[co] = psx
        if co == 1:
            emit_attnv_single(pend[-1]["p"], exs.pop(0), pend[-1]["ps_y2"])
            pend[-1]["r65s"] = emit_recips(pend[-1]["p"], pend[-1]["ps_y2"])
    emit_deferred_epilogue(last=True, **pend.pop())
    if "y" in dbg:
        for c in range(C6):
            dy_ = P4.tile([128, 512], F32, tag="dbgt", name=f"dby{c}", bufs=1)
            nc.vector.tensor_copy(out=dy_, in_=y_sb[:, c, :])
            nc.sync.dma_start(out=dbg["y"][ts(c, 128), :], in_=dy_)
    nc.tensor.matmul(pp0[0], wproj_sb[:, 5, ts(0, 128)], y_sb[:, 5, :],
                     start=False, stop=True)
    nc.tensor.matmul(ps_us[0], uproj_s[:, 5:6], y_sb[:, 5, :],
                     start=False, stop=True)
    for co in (1, 2):
        nc.tensor.matmul(pc12[co], wproj_sb[:, 5, ts(co, 128)], y_sb[:, 5, :],
                         start=False, stop=True)
    nc.vector.scalar_tensor_tensor(out=x2_sb[:, 0, :], in0=pp0[0],
                                   scalar=bproj_s[:, 0:1],
                                   in1=xq_sb[:, 0, :], op0=OP.add, op1=OP.add)
    # squares on ACT (idle after the last exp; Square lives in every act
    # table set). LN2 row chain part 1 interleaves on ACT after sq0.
    sq0 = P4.tile([128, 512], BF16, tag="tmp", name="sq2_0")
    nc.scalar.activation(sq0, x2_sb[:, 0, :], AF.Square)
    srow = P6.tile([1, 512], F32, tag="r", bufs=4)
    nc.vector.scalar_tensor_tensor(out=srow, in0=ps_us[0], scalar=float(sbp),
                                   in1=sumx0, op0=OP.add, op1=OP.add)
    mrow2 = P6.tile([1, 512], BF16, tag="r", bufs=4)
    nc.scalar.activation(mrow2, srow, AF.Copy, scale=1.0 / D)
    m22 = P6.tile([1, 512], F32, tag="r", bufs=4)
    nc.scalar.activation(m22, srow, AF.Square, scale=1.0 / D)
    mbs2 = P4.tile([128, 512], BF16, tag="bcs", bufs=8)
    nc.gpsimd.partition_broadcast(mbs2, mrow2)
    d2l = [None] * C6
    sql = [sq0]
    ps_sq2 = ps_mm.tile([1, 512], F32, tag="mm")
    for co in (1, 2):
        nc.vector.scalar_tensor_tensor(out=x2_sb[:, co, :], in0=pc12[co],
                                       scalar=bproj_s[:, co:co + 1],
                                       in1=xq_sb[:, co, :],
                                       op0=OP.add, op1=OP.add)
        sq = P4.tile([128, 512], BF16, tag="tmp", name=f"sq2_{co}")
        nc.scalar.activation(sq, x2_sb[:, co, :], AF.Square)
        sql.append(sq)
    # remaining proj chunks; per-chunk: residual epilogue, square, and the
    # sumsq accumulation (behind by one, so PE never waits ACT)
    for co in range(3, C6):
        ps = ps_mm.tile([128, 512], F32, tag="mm")
        for c in range(C6):
            nc.tensor.matmul(ps, wproj_sb[:, c, ts(co, 128)], y_sb[:, c, :],
                             start=(c == 0), stop=(c == C6 - 1))
        nc.tensor.matmul(ps_sq2, ones_col, sql[co - 3],
                         start=(co == 3), stop=False)
        nc.vector.scalar_tensor_tensor(out=x2_sb[:, co, :], in0=ps,
                                       scalar=bproj_s[:, co:co + 1],
                                       in1=xq_sb[:, co, :],
                                       op0=OP.add, op1=OP.add)
        sq = P4.tile([128, 512], BF16, tag="tmp", name=f"sq2_{co}")
        nc.scalar.activation(sq, x2_sb[:, co, :], AF.Square)
        sql.append(sq)
        c = co - 3
        d2l[c] = P4.tile([128, 512], BF16, tag="bcs", name=f"d2_{c}", bufs=8)
        nc.vector.tensor_tensor(d2l[c], x2_sb[:, c, :], mbs2, op=OP.subtract)
    for c in range(3, C6):
        nc.tensor.matmul(ps_sq2, ones_col, sql[c],
                         start=False, stop=(c == C6 - 1))

    if "x2" in dbg:
        for c in range(C6):
            dx2_ = P4.tile([128, 512], F32, tag="dbgt", name=f"dbx2{c}", bufs=1)
            nc.vector.tensor_copy(out=dx2_, in_=x2_sb[:, c, :])
            nc.sync.dma_start(out=dbg["x2"][ts(c, 128), :], in_=dx2_)
    # ========== LN2 rest of row chain ==========
    vrow2 = P6.tile([1, 512], F32, tag="r", bufs=4)
    nc.vector.scalar_tensor_tensor(out=vrow2, in0=ps_sq2, scalar=1.0 / D,
                                   in1=m22, op0=OP.mult, op1=OP.subtract)
    srt2 = P6.tile([1, 512], F32, tag="r", bufs=4)
    nc.scalar.activation(srt2, vrow2, AF.Sqrt, bias=eps1)
    warm2 = P6.tile([1, 512], F32, tag="r", bufs=4, name="warm2")
    nc.scalar.activation(warm2[0:1, 0:8], srt2[0:1, 0:8], AF.Gelu)
    rf2 = P6.tile([1, 512], F32, tag="r", bufs=4)
    nc.vector.reciprocal_approx_fast(out=rf2, in_=srt2)
    rrow2 = P6.tile([1, 512], BF16, tag="r", bufs=4)
    nc.vector.tensor_copy(out=rrow2, in_=rf2)
    for c in (3, 4, 5):
        d2l[c] = P4.tile([128, 512], BF16, tag="bcs", name=f"d2_{c}", bufs=8)
        nc.vector.tensor_tensor(d2l[c], x2_sb[:, c, :], mbs2, op=OP.subtract)
    rbs2 = P4.tile([128, 512], BF16, tag="bcs", bufs=8)
    nc.gpsimd.partition_broadcast(rbs2, rrow2)
    for c in range(C6):
        nc.vector.tensor_tensor(xn2_sb[:, c, :], d2l[c], rbs2, op=OP.mult)

    # ========== MLP (fc1 fp8 DoubleRow, fc2 bf16) ==========
    h_sb = P1.tile([128, HO24, NQ], BF16, tag="h")
    for ho in range(HO24):
        ps = ps_mm.tile([128, 512], F32, tag="mm")
        for ci in range(0, C6, 2):
            nc.tensor.matmul(ps, wfc1_sb[:, ci:ci + 2, ts(ho, 128)],
                             xn2_sb[:, ci:ci + 2, :],
                             start=(ci == 0), stop=(ci == C6 - 2), perf_mode=DR)
        nc.scalar.activation(h_sb[:, ho, :], ps, AF.Gelu,
                             scale=1.0 / WS, bias=bfc1_s[:, ho:ho + 1])
    for co in range(C6):
        ps = ps_mm.tile([128, 512], F32, tag="mm")
        for ho in range(HO24):
            wsrc = wfc2a if ho < 12 else wfc2b
            nc.tensor.matmul(ps, wsrc[:, ho % 12, ts(co, 128)], h_sb[:, ho, :],
                             start=(ho == 0), stop=(ho == HO24 - 1))
        o = P2.tile([128, 512], F32, tag="ot", name=f"o_{co}")
        nc.vector.scalar_tensor_tensor(out=o, in0=ps,
                                       scalar=bfc2_s[:, co:co + 1],
                                       in1=x2_sb[:, co, :],
                                       op0=OP.add, op1=OP.add)
        nc.sync.dma_start(out=out_d[ts(co, 128), :], in_=o)

    for cm in (dr_cm, ps_sc_cm, ps_mm_cm, P6_cm, P4_cm, P2_cm, P1_cm):
        cm.__exit__(None, None, None)


def _host_prep(x, mask, ln1_g, ln1_b, qkv_w, proj_w, proj_b, ln2_g, ln2_b,
               fc1_w, fc1_b, fc2_w, fc2_b):
    bf = ml_dtypes.bfloat16
    f8 = ml_dtypes.float8_e4m3
    f32 = np.float32
    x = np.asarray(x, f32)
    mask = np.asarray(mask)
    qkv_w = np.asarray(qkv_w, f32)
    proj_w = np.asarray(proj_w, f32)
    fc1_w = np.asarray(fc1_w, f32)
    fc2_w = np.asarray(fc2_w, f32)
    ln1_g = np.asarray(ln1_g, f32); ln1_b = np.asarray(ln1_b, f32)
    ln2_g = np.asarray(ln2_g, f32); ln2_b = np.asarray(ln2_b, f32)
    proj_b = np.asarray(proj_b, f32)
    fc1_b = np.asarray(fc1_b, f32); fc2_b = np.asarray(fc2_b, f32)

    wqkv_f = qkv_w * ln1_g[None, :]
    bqkv_f = qkv_w @ ln1_b
    wqkv_f[0:D] *= SCALE
    bqkv_f[0:D] *= SCALE
    bv = bqkv_f[2 * D:3 * D].copy()
    bqkv_f[2 * D:3 * D] = 0.0     # v bias folded into proj bias (sum(attn)=1)
    bproj_f = proj_b + proj_w @ bv
    wfc1_f = fc1_w * ln2_g[None, :]
    bfc1_f = fc1_w @ ln2_b + fc1_b

    shared = {
        "wqkv": np.ascontiguousarray(wqkv_f.T * WS).astype(f8),
        "wproj": np.ascontiguousarray(proj_w.T).astype(bf),
        "wfc1": np.ascontiguousarray(wfc1_f.T * WS).astype(f8),
        "wfc2": np.ascontiguousarray(fc2_w.T).astype(bf),
        "bqkv": np.ascontiguousarray(bqkv_f.reshape(18, 128).T).astype(f32),
        "bproj": np.ascontiguousarray(bproj_f.reshape(6, 128).T).astype(f32),
        "bfc1": np.ascontiguousarray(bfc1_f.reshape(24, 128).T).astype(f32),
        "bfc2": np.ascontiguousarray(fc2_b.reshape(6, 128).T).astype(f32),
        "uproj": np.ascontiguousarray(
            proj_w.sum(axis=0).reshape(6, 128).T).astype(bf),
    }
    sbp = float(bproj_f.sum())

    # per-batch compacted key set (host-side gather of unmasked tokens)
    xk_b, m01_b = [], []
    for b in range(B):
        idx = np.nonzero(mask[b] != 1)[0]
        nk = len(idx)
        assert nk <= NK, f"batch {b}: {nk} unmasked keys > NK={NK}"
        xk = np.zeros((NK, D), f32)
        xk[:nk] = x[b][idx]
        m01 = np.zeros((NK,), f32)
        m01[:nk] = WS   # cancels the WS carried by the V weights
        xk_b.append(np.ascontiguousarray(xk.T).astype(bf))
        m01_b.append(np.ascontiguousarray(m01.reshape(K9, 128).T).astype(f32))

    in_maps = []
    for core in range(NC):
        b, s = divmod(core, NSH)
        im = dict(shared)
        im["xqT"] = np.ascontiguousarray(
            x[b, s * NQ:(s + 1) * NQ].T).astype(bf)
        im["xkT"] = xk_b[b]
        im["mask01"] = m01_b[b]
        in_maps.append(im)
    return in_maps, sbp


def kernel(**inputs):
    in_maps, sbp = _host_prep(**inputs)
    if _cached.get("sbp") != sbp:
        _cached["nc"] = _build_nc(sbp)
        _cached["sbp"] = sbp
    res = run_bass_kernel_spmd(_cached["nc"], in_maps, core_ids=list(range(NC)))
    out = np.empty((B, N, D), np.float32)
    for core in range(NC):
        b, s = divmod(core, NSH)
        out[b, s * NQ:(s + 1) * NQ, :] = res.results[core]["out"].T
    return out
